# revision 32
# baseline (speedup 1.0000x reference)
"""Trainium2 Bass kernel for GQA multi-head attention (nn_MultiHeadAttention).

Reference computation (fp32):
    q = h @ Wq^T -> RoPE ; k = h @ Wk^T -> RoPE ; v = h @ Wv^T
    scores = q k^T / sqrt(64) + causal_mask ; w = softmax(scores)
    out = (w v) @ Wo^T

Shapes: h [2,2048,2048], Wq [2048,2048], Wk/Wv [512,2048], Wo [2048,2048],
32 q heads / 8 kv heads (GQA group=4), head_dim 64.

Sharding: tensor-parallel over the 8 kv-head groups, one group per core.
Core g owns q heads [4g,4g+4), kv head g, Wo columns [256g, 256(g+1)).
Each core computes a full-token partial of the output projection; the host
sums the 8 partials (the Wo contraction splits over head blocks).

Structure: a single software-pipelined loop over the 8 token tiles of 512;
at step s it emits [hT prefetch for s+1] [attention for query tile s-1]
[QKV projection + RoPE for tile s] [Wo projection + output store for tile
s-2].  The 2-step Wo skew means the Wo matmuls' inputs are always
long-ready (they fill PE gaps so the HAM clock gate stays warm, and their
PSUM-evacuation copies never head-of-line-block the exps), and the softmax
normalize chain of tile s-1 gets a full step of slack before anything
consumes it.  Engine assignment keeps each FIFO's latency-critical ops
away from bulk work: PE matmuls; ACT exps + a slice of the PSUM
evacuations; DVE RoPE muls/adds + evacuation copies + reciprocal; GPSIMD
causal masks only; the normalize partition-broadcast is a stride-0 DMA
through a DRAM scratch (no compute engine involved).

Everything is kept transposed: h^T [2048, 4096] comes in, Q^T/K^T [d, t]
fall out of the projections directly, scores are S^T[k, q], softmax is a
plain exp (scores are O(5), fp32-safe) with causal-skip at 128-key-block
granularity AND 128-query-column truncation inside the diagonal straddle
blocks (scores / exp / mask / A@V all skip the dead triangle).  A@V uses V
augmented with a ones-column so softmax denominators fall out of the same
matmul.  RoPE's rot_half partition swap runs on a pre-scaled copy
(z = x * sin_pre, then swap z via SBUF DMA) so no scalar-engine staging
copy is needed; the odd-head K replica is a second (partition-shifted)
DVE add instead of a DMA.  Denominator reciprocals for all 4 heads of a
query tile are batched through one [32, 64] bounce so the DVE reciprocal
uses 32 lanes.  Output partials are stored bf16 (halves the HBM write) as
one coalesced DMA per query tile.
"""

import sys

for _p in ("/opt/trn_rl_repo",):
    if _p not in sys.path:
        sys.path.insert(0, _p)

import numpy as np
import ml_dtypes

D = 2048          # model dim
HD = 64           # head dim
S = 2048          # sequence
B = 2             # batch
T = B * S         # total tokens
EQ = 256          # q-projection rows per core (4 heads x 64)
TT = 512          # token tile (both projection and query tile)
NT = T // TT      # 8 merged steps
NDB = D // 128    # contraction blocks for projections
QT = 512          # query tile for attention
KBLK = 128        # key block for attention
BF16 = ml_dtypes.bfloat16

_CACHE = {}


def _build_program(causal: bool):
    """Build the single-core Bass/Tile program (identical across cores)."""
    import concourse.bass as bass
    import concourse.mybir as mybir
    import concourse.tile as tile
    from concourse import bacc
    from concourse.masks import make_identity

    f32 = mybir.dt.float32
    bf16 = mybir.dt.bfloat16
    f8 = mybir.dt.float8e4

    nc = bacc.Bacc("TRN2", target_bir_lowering=False, debug=False)

    hT = nc.dram_tensor("hT", [D, T], bf16, kind="ExternalInput").ap()
    wqT = nc.dram_tensor("wqT", [D, EQ], bf16, kind="ExternalInput").ap()
    # k and v projection weights packed [D, 64+64] so one matmul produces both
    wkvT = nc.dram_tensor("wkvT", [D, 2 * HD], bf16, kind="ExternalInput").ap()
    woT = nc.dram_tensor("woT", [EQ, D], bf16, kind="ExternalInput").ap()
    cos2 = nc.dram_tensor("cos2", [128, T], f32, kind="ExternalInput").ap()
    # sin with rot_half sign AND partition swap pre-applied (see _host_inputs)
    sinp = nc.dram_tensor("sinp", [128, T], f32, kind="ExternalInput").ap()
    # mask^T tiles, only used when causal=False
    maskT = nc.dram_tensor("maskT", [S, S], f32, kind="ExternalInput").ap()
    outT = nc.dram_tensor("outT", [D, T], bf16, kind="ExternalOutput").ap()

    hT_b3 = hT.rearrange("(n p) t -> p n t", p=128)     # [128, 16, T]
    wqT_b = wqT.rearrange("(n p) e -> p n e", p=128)
    wkvT_b = wkvT.rearrange("(n p) e -> p n e", p=128)
    woT_b = woT.rearrange("(n p) e -> p n e", p=128)
    outT_b3 = outT.rearrange("(n p) t -> p n t", p=128)  # [128, 16, T]

    Exp = mybir.ActivationFunctionType.Exp
    PSUM = bass.MemorySpace.PSUM

    with tile.TileContext(nc) as tc:
        import contextlib

        with contextlib.ExitStack() as stack:
            const = stack.enter_context(tc.tile_pool(name="const", bufs=1))

            wq_s = const.tile([128, NDB, EQ], bf16)
            wkv_s = const.tile([128, NDB, 2 * HD], bf16)
            wo_s = const.tile([128, 2, D], bf16)
            cos_s = const.tile([128, T], f32)
            sinp_s = const.tile([128, T], f32)
            qt_s = [
                const.tile([128, T], bf16, tag=f"qt{i}", name=f"qt{i}")
                for i in range(2)
            ]
            kt_s = const.tile([128, T], bf16)
            va_s = const.tile([128, T // 128, HD + 1], bf16)
            tri_s = const.tile([128, 4, QT], bf16)
            ident = const.tile([128, 128], f32)

            nc.sync.dma_start(out=wq_s, in_=wqT_b)
            nc.sync.dma_start(out=wkv_s, in_=wkvT_b)
            # cos/sin/wo are not needed until RoPE / step 2 -- load them on
            # the scalar HWDGE ring so they don't delay the first hT
            nc.scalar.dma_start(out=cos_s, in_=cos2)
            nc.scalar.dma_start(out=sinp_s, in_=sinp)
            nc.scalar.dma_start(out=wo_s, in_=woT_b)

            make_identity(nc, ident)
            # ones column of the augmented V
            nc.gpsimd.memset(va_s[:, :, HD : HD + 1], 1.0)
            # multiplicative causal masks for the 4 straddle offsets:
            # tri_s[p, j, f] = 1.0 where f >= p + 128*j else 0.0
            for j in range(4):
                nc.gpsimd.memset(tri_s[:, j, :], 1.0)
                nc.gpsimd.affine_select(
                    out=tri_s[:, j, :],
                    in_=tri_s[:, j, :],
                    compare_op=mybir.AluOpType.is_ge,
                    fill=0.0,
                    base=-128 * j,
                    channel_multiplier=-1,
                    pattern=[[1, QT]],
                )

            # ---------------- pools for the merged pipeline
            with contextlib.ExitStack() as pp:
                ht_pool = pp.enter_context(tc.tile_pool(name="ht", bufs=2))
                # m1 / z / swapped-z rope scratch, all three ropes stacked
                rp_pool = pp.enter_context(tc.tile_pool(name="rp", bufs=1))
                vs_pool = pp.enter_context(tc.tile_pool(name="vs", bufs=2))
                # shared-PSUM pool: proj accumulators, V transposes, Wo tiles
                ps_mm = pp.enter_context(
                    tc.tile_pool(name="ps_mm", bufs=2, space=PSUM)
                )
                ps_s = pp.enter_context(
                    tc.tile_pool(name="ps_s", bufs=2, space=PSUM)
                )
                ps_o = pp.enter_context(
                    tc.tile_pool(name="ps_o", bufs=1, space=PSUM)
                )
                pt_pool = pp.enter_context(tc.tile_pool(name="pt", bufs=4))
                on_pool = pp.enter_context(tc.tile_pool(name="on", bufs=2))
                nm_pool = pp.enter_context(tc.tile_pool(name="nm", bufs=1))
                oa_pool = pp.enter_context(tc.tile_pool(name="oa", bufs=1))
                dr_pool = pp.enter_context(
                    tc.tile_pool(name="dr", bufs=2, space="DRAM")
                )

                def prefetch(it):
                    htile = ht_pool.tile([128, NDB, TT], bf16, tag="ht",
                                         name=f"ht{it}")
                    t0 = it * TT
                    nc.sync.dma_start(out=htile, in_=hT_b3[:, :, t0 : t0 + TT])
                    return htile

                def proj_chunk(it, htile, ri, state):
                    """One projection group (q01 / q23 / kv) + its RoPE muls."""
                    t0 = it * TT
                    tsl = slice(t0, t0 + TT)
                    if ri == 0:
                        state["m1"] = rp_pool.tile([128, 3, TT], f32, tag="m1", name="m1")
                        state["z"] = rp_pool.tile([128, 3, TT], f32, tag="z", name="z")
                        state["m2p"] = rp_pool.tile([128, 3, TT], f32, tag="m2p", name="m2p")
                    m1_all, z_all = state["m1"], state["z"]
                    wsrc, e0, e1, nrows = [
                        (wkv_s, 0, 2 * HD, 64),
                        (wq_s, 0, 128, 128),
                        (wq_s, 128, 256, 128),
                    ][ri]
                    ps = ps_mm.tile([128, TT], f32, tag="mm2k", name=f"pj{ri}")
                    for idb in range(NDB):
                        nc.tensor.matmul(
                            ps,
                            wsrc[:, idb, e0:e1],
                            htile[:, idb, :],
                            start=(idb == 0),
                            stop=(idb == NDB - 1),
                        )
                    # RoPE input products; m2p (swapped z) comes via DMA
                    nc.vector.tensor_mul(
                        m1_all[:nrows, ri, :], ps[:nrows], cos_s[:nrows, tsl]
                    )
                    nc.vector.tensor_mul(
                        z_all[:nrows, ri, :], ps[:nrows], sinp_s[:nrows, tsl]
                    )
                    if ri == 0:
                        # stage V to SBUF immediately: the next-but-one
                        # mm2k allocation reuses this PSUM slot
                        v_sb = vs_pool.tile([128, TT], f32, tag="v_sb")
                        nc.scalar.copy(out=v_sb[64:128, :], in_=ps[64:128, :])
                        state["v_sb"] = v_sb

                def proj_tail(it, state):
                    """Swap-DMA + RoPE adds + V transpose for token tile it."""
                    t0 = it * TT
                    tsl = slice(t0, t0 + TT)
                    m1_all, z_all, m2p_all = state["m1"], state["z"], state["m2p"]
                    # partition swap of z (32-row block pairs 0<->1, 2<->3)
                    for c, lo in ((0, 32), (1, 0), (2, 96), (3, 64)):
                        nc.sync.dma_start(
                            out=m2p_all[c * 32 : c * 32 + 32, :, :],
                            in_=z_all[lo : lo + 32, :, :],
                        )
                    # rope adds; k lands twice so odd q-heads can matmul
                    # from partition base 64 (tile_position row packing).
                    # slot 0 = kv, slots 1/2 = the two q head-pairs
                    nc.vector.tensor_add(
                        kt_s[0:64, tsl], m1_all[0:64, 0, :], m2p_all[0:64, 0, :]
                    )
                    nc.vector.tensor_add(
                        kt_s[64:128, tsl], m1_all[0:64, 0, :], m2p_all[0:64, 0, :]
                    )
                    nc.vector.tensor_add(
                        qt_s[0][:, tsl], m1_all[:, 1, :], m2p_all[:, 1, :]
                    )
                    nc.vector.tensor_add(
                        qt_s[1][:, tsl], m1_all[:, 2, :], m2p_all[:, 2, :]
                    )
                    # V: [d, t] -> [t, d] via PE transpose (V was staged
                    # to SBUF right after the kv projection)
                    v_sb = state["v_sb"]
                    for c4 in range(TT // 128):
                        vt_ps = ps_mm.tile([128, HD], f32, tag="mm2k", name="vt")
                        nc.tensor.transpose(
                            vt_ps,
                            v_sb[64:128, c4 * 128 : (c4 + 1) * 128],
                            ident[64:128, 64:128],
                        )
                        nc.vector.tensor_copy(
                            out=va_s[:, it * 4 + c4, 0:HD], in_=vt_ps
                        )

                def attn_block(it, astate, rp, kb, nkb):
                    """One 128-key attention block of query tile it."""
                    b, iq = it // 4, it % 4
                    q0 = iq * QT
                    bq = b * S + q0
                    qtile = qt_s[rp]
                    if kb == 0:
                        # 2 PSUM banks shared by both head-pairs: rp1's first
                        # A@V slot-waits on rp0's PSUM evacuation copies
                        astate[f"o{rp}"] = [
                            ps_o.tile(
                                [65, QT], f32, tag=f"o{i}", name=f"o{i}", bufs=1
                            )
                            for i in range(2)
                        ]
                    o_ps = astate[f"o{rp}"]
                    ksl = slice(b * S + kb * KBLK, b * S + (kb + 1) * KBLK)
                    j = kb - q0 // KBLK
                    # query-column truncation: straddle block j only
                    # touches queries f >= 128*j
                    c0 = 128 * j if (causal and j > 0) else 0
                    s_ps = ps_s.tile([128, 2, QT], f32, tag="s")
                    pt = pt_pool.tile([128, 2, QT], bf16, tag="pt")
                    for h in range(2):
                        hb = h * 64
                        nc.tensor.matmul(
                            s_ps[:, h, c0:QT],
                            kt_s[hb : hb + 64, ksl],
                            qtile[hb : hb + 64, bq + c0 : bq + QT],
                            start=True,
                            stop=True,
                        )
                    if causal:
                        nc.scalar.activation(
                            pt[:, :, c0:QT], s_ps[:, :, c0:QT], Exp, scale=0.125
                        )
                    else:
                        mk = pt_pool.tile([128, QT], f32, tag="mk")
                        sm = pt_pool.tile([128, 2, QT], f32, tag="sm")
                        nc.sync.dma_start(
                            out=mk,
                            in_=maskT[kb * KBLK : (kb + 1) * KBLK, q0 : q0 + QT],
                        )
                        for h in range(2):
                            nc.vector.scalar_tensor_tensor(
                                out=sm[:, h, :],
                                in0=s_ps[:, h, :],
                                scalar=0.125,
                                in1=mk,
                                op0=mybir.AluOpType.mult,
                                op1=mybir.AluOpType.add,
                            )
                        nc.scalar.activation(pt, sm, Exp, scale=1.0)
                    for h in range(2):
                        if causal and j >= 0:
                            # zero the sub-diagonal triangle in place on the
                            # (otherwise idle) gpsimd: keep where (f-c0) >= p
                            nc.gpsimd.affine_select(
                                out=pt[:, h, c0:QT],
                                in_=pt[:, h, c0:QT],
                                compare_op=mybir.AluOpType.is_ge,
                                fill=0.0,
                                base=0,
                                channel_multiplier=-1,
                                pattern=[[1, QT - c0]],
                            )
                        nc.tensor.matmul(
                            o_ps[h][:, c0:QT],
                            va_s[:, b * (S // 128) + kb, :],
                            pt[:, h, c0:QT],
                            start=(kb == 0),
                            stop=(kb == nkb - 1),
                        )

                def evac_rp(astate, rp):
                    """Evacuate the pair's A@V accumulators (frees the two
                    o PSUM banks for the next head pair)."""
                    ou_all = astate["ou"]
                    o_ps = astate[f"o{rp}"]
                    for h in range(2):
                        nc.vector.tensor_copy(
                            out=ou_all[:, rp * 2 + h, :], in_=o_ps[h]
                        )

                def normalize_tail(astate):
                    """Batched softmax normalization for all 4 heads: the
                    denominator rows bounce through a [32, 64] layout so
                    reciprocal uses 32 lanes, and the partition broadcast is
                    a stride-0 DMA through a DRAM scratch (no engine time)."""
                    on_t, ou_all = astate["on_t"], astate["ou"]
                    r32 = nm_pool.tile([32, 64], f32, tag="r32")
                    nc.sync.dma_start(out=r32, in_=ou_all[64:65, :, :])
                    r32r = nm_pool.tile([32, 64], f32, tag="r32r")
                    nc.vector.reciprocal(r32r, r32)
                    rd = dr_pool.tile([1, 4 * QT], f32, tag="rd", name="rd")
                    nc.sync.dma_start(out=rd, in_=r32r)
                    rec_b = nm_pool.tile([64, 4 * QT], f32, tag="rb")
                    nc.sync.dma_start(
                        out=rec_b, in_=rd.partition_broadcast(64)[:, 0, :]
                    )
                    for rp in range(2):
                        for h in range(2):
                            hh = rp * 2 + h
                            nc.vector.tensor_mul(
                                on_t[rp][h * 64 : h * 64 + 64, :],
                                ou_all[0:64, hh, :],
                                rec_b[:, hh * QT : (hh + 1) * QT],
                            )

                def attn_begin(it):
                    b, iq = it // 4, it % 4
                    nkb = (iq * QT // KBLK + 4) if causal else (S // KBLK)
                    astate = {
                        "on_t": [
                            on_pool.tile(
                                [128, QT], bf16, tag=f"on{i}", name=f"on{i}"
                            )
                            for i in range(2)
                        ],
                        "ou": nm_pool.tile([65, 4, QT], f32, tag="ou", name="ou", bufs=2),
                        "nkb": nkb,
                    }
                    return astate

                def attn_out(it, on_t):
                    """Wo projection + coalesced bf16 output store."""
                    b, iq = it // 4, it % 4
                    q0 = iq * QT
                    qsl = slice(b * S + q0, b * S + q0 + QT)
                    out_acc = oa_pool.tile([128, D // 128, QT], bf16, tag="oacc")
                    for eb in range(D // 128):
                        wo_ps = ps_mm.tile([128, QT], f32, tag="mm2k", name="wo")
                        for db in range(2):
                            nc.tensor.matmul(
                                wo_ps,
                                wo_s[:, db, eb * 128 : (eb + 1) * 128],
                                on_t[db],
                                start=(db == 0),
                                stop=(db == 1),
                            )
                        # split PSUM evacuation: mostly DVE, a bit on ACT
                        if eb % 3 == 1:
                            nc.scalar.copy(out=out_acc[:, eb, :], in_=wo_ps)
                        else:
                            nc.vector.tensor_copy(
                                out=out_acc[:, eb, :], in_=wo_ps
                            )
                    half = D // 256
                    nc.sync.dma_start(
                        out=outT_b3[:, 0:half, qsl], in_=out_acc[:, 0:half, :]
                    )
                    nc.sync.dma_start(
                        out=outT_b3[:, half:, qsl], in_=out_acc[:, half:, :]
                    )

                # ---------------- the software-pipelined merged loop:
                # attention for tile s-1, projection for tile s, Wo for tile
                # s-2.  The 2-step Wo skew means the Wo matmuls' inputs are
                # always long-ready (they fill PE gaps, and their PSUM
                # evacuation copies never head-of-line-block the exps), and
                # the normalize chain of s-1 has a full step to complete.
                htiles = {0: prefetch(0)}
                on_hist = {}
                for step in range(NT + 2):
                    if step + 1 <= NT - 1:
                        htiles[step + 1] = prefetch(step + 1)
                    if 1 <= step <= NT:
                        astate = attn_begin(step - 1)
                        nkb = astate["nkb"]
                        for rp in range(2):
                            for kb in range(nkb):
                                attn_block(step - 1, astate, rp, kb, nkb)
                            evac_rp(astate, rp)
                        normalize_tail(astate)
                        on_hist[step - 1] = astate["on_t"]
                    if step <= NT - 1:
                        pstate = {}
                        htile = htiles.pop(step)
                        for ri in range(3):
                            proj_chunk(step, htile, ri, pstate)
                        proj_tail(step, pstate)
                    if step >= 2:
                        attn_out(step - 2, on_hist.pop(step - 2))

    nc.compile()
    return nc


def _host_inputs(inputs, causal):
    """Shard + transpose the full inputs into 8 per-core input maps."""
    h = np.asarray(inputs["hidden_states"], np.float32)
    cos = np.asarray(inputs["position_cos"], np.float32)
    sin = np.asarray(inputs["position_sin"], np.float32)
    Wq = np.asarray(inputs["Wq"], np.float32)
    Wk = np.asarray(inputs["Wk"], np.float32)
    Wv = np.asarray(inputs["Wv"], np.float32)
    Wo = np.asarray(inputs["Wo"], np.float32)
    mask = np.asarray(inputs["attention_mask"], np.float32)[0, 0]

    hT = np.ascontiguousarray(h.reshape(T, D).T).astype(BF16)

    cosT = np.tile(cos.T, (1, B))                     # [64, T]
    sinT = np.tile(sin.T, (1, B))
    cos2 = np.ascontiguousarray(np.vstack([cosT, cosT]).astype(np.float32))
    s_signed = np.vstack([-sinT[0:32], sinT[32:64]])  # rot_half sign baked in
    sin2s = np.vstack([s_signed, s_signed])           # [128, T]
    # pre-swap so that z[p] = x[p]*sinp[p]; m2[p] = z[swap(p)] equals
    # rot_half(x)[p] * sin_signed[p]  (swap = 32-row block pairs 0<->1, 2<->3)
    swap_idx = np.concatenate(
        [np.arange(32, 64), np.arange(0, 32), np.arange(96, 128), np.arange(64, 96)]
    )
    sinp = np.ascontiguousarray(sin2s[swap_idx].astype(np.float32))

    maskT = np.ascontiguousarray(mask.T).astype(np.float32)

    in_maps = []
    for g in range(8):
        in_maps.append(
            {
                "hT": hT,
                "wqT": np.ascontiguousarray(
                    Wq[g * EQ : (g + 1) * EQ].T
                ).astype(BF16),
                "wkvT": np.ascontiguousarray(
                    np.concatenate(
                        [
                            Wk[g * HD : (g + 1) * HD].T,
                            Wv[g * HD : (g + 1) * HD].T,
                        ],
                        axis=1,
                    )
                ).astype(BF16),
                "woT": np.ascontiguousarray(
                    Wo[:, g * EQ : (g + 1) * EQ].T
                ).astype(BF16),
                "cos2": cos2,
                "sinp": sinp,
                "maskT": maskT,
            }
        )
    return in_maps


def _is_causal(mask):
    m = np.asarray(mask, np.float32)[0, 0]
    tri = np.tril(np.ones((S, S), bool))
    return bool(np.all(m[tri] == 0.0) and np.all(m[~tri] <= -1e8))


def _assemble(results):
    acc = np.zeros((D, T), np.float32)
    for r in results:
        acc += r["outT"].astype(np.float32)
    return np.ascontiguousarray(acc.reshape(D, B, S).transpose(1, 2, 0))


def kernel(**inputs) -> np.ndarray:
    from concourse.bass_utils import run_bass_kernel_spmd

    causal = _is_causal(inputs["attention_mask"])
    key = ("prog", causal)
    if key not in _CACHE:
        _CACHE[key] = _build_program(causal)
    nc = _CACHE[key]

    in_maps = _host_inputs(inputs, causal)
    res = run_bass_kernel_spmd(nc, in_maps, core_ids=list(range(8)))
    return _assemble(res.results)


# revision 33
# speedup vs baseline: 1.1464x; 1.1464x over previous
"""Trainium2 Bass kernel for GQA multi-head attention (nn_MultiHeadAttention).

Reference computation (fp32):
    q = h @ Wq^T -> RoPE ; k = h @ Wk^T -> RoPE ; v = h @ Wv^T
    scores = q k^T / sqrt(64) + causal_mask ; w = softmax(scores)
    out = (w v) @ Wo^T

Shapes: h [2,2048,2048], Wq [2048,2048], Wk/Wv [512,2048], Wo [2048,2048],
32 q heads / 8 kv heads (GQA group=4), head_dim 64.

Sharding: tensor-parallel over the 8 kv-head groups, one group per core.
Core g owns q heads [4g,4g+4), kv head g, Wo columns [256g, 256(g+1)).
Each core computes a full-token partial of the output projection; the host
sums the 8 partials (the Wo contraction splits over head blocks).

v3 structure (single software-pipelined loop over 8 token tiles of 512):
    step s:  [hT prefetch for s+1] [QKV proj + RoPE for tile s]
             [attention for query tile s-1] [Wo + output store for s-1]
so every engine has dense interleaved work and the PE never idles long
enough for the HAM clock gate to re-throttle.  Engine assignment is
balanced: PE matmuls, ACT exps + PSUM evacuations, DVE RoPE + copies,
GPSIMD causal masks + normalize broadcast/muls.

Everything is kept transposed: h^T [2048, 4096] comes in, Q^T/K^T [d, t]
fall out of the projections directly, scores are S^T[k, q], softmax is a
plain exp (scores are O(5), fp32-safe) with causal-skip at 128-key-block
granularity AND 128-query-column truncation inside the diagonal straddle
blocks (scores / exp / mask / A@V all skip the dead triangle).  A@V uses V
augmented with a ones-column so softmax denominators fall out of the same
matmul.  RoPE's rot_half partition swap runs on a pre-scaled copy
(z = x * sin_pre, then swap z via SBUF DMA) so no scalar-engine staging
copy is needed; the odd-head K replica is a second (partition-shifted)
DVE add instead of a DMA.  Denominator reciprocals for all 4 heads of a
query tile are batched through one [32, 64] bounce so the DVE reciprocal
uses 32 lanes.  Output partials are stored bf16 (halves the HBM write) as
one coalesced DMA per query tile.
"""

import sys

for _p in ("/opt/trn_rl_repo",):
    if _p not in sys.path:
        sys.path.insert(0, _p)

import numpy as np
import ml_dtypes

D = 2048          # model dim
HD = 64           # head dim
S = 2048          # sequence
B = 2             # batch
T = B * S         # total tokens
EQ = 256          # q-projection rows per core (4 heads x 64)
TT = 512          # token tile (both projection and query tile)
NT = T // TT      # 8 merged steps
NDB = D // 128    # contraction blocks for projections
QT = 512          # query tile for attention
KBLK = 128        # key block for attention
BF16 = ml_dtypes.bfloat16

_CACHE = {}


def _build_program(causal: bool):
    """Build the single-core Bass/Tile program (identical across cores)."""
    import concourse.bass as bass
    import concourse.mybir as mybir
    import concourse.tile as tile
    from concourse import bacc
    from concourse.masks import make_identity

    f32 = mybir.dt.float32
    bf16 = mybir.dt.bfloat16
    f8 = mybir.dt.float8e4

    nc = bacc.Bacc("TRN2", target_bir_lowering=False, debug=False)

    hT = nc.dram_tensor("hT", [D, T], bf16, kind="ExternalInput").ap()
    wqT = nc.dram_tensor("wqT", [D, EQ], bf16, kind="ExternalInput").ap()
    # k and v projection weights packed [D, 64+64] so one matmul produces both
    wkvT = nc.dram_tensor("wkvT", [D, 2 * HD], bf16, kind="ExternalInput").ap()
    woT = nc.dram_tensor("woT", [EQ, D], bf16, kind="ExternalInput").ap()
    cos2 = nc.dram_tensor("cos2", [128, T], f32, kind="ExternalInput").ap()
    # sin with rot_half sign AND partition swap pre-applied (see _host_inputs)
    sinp = nc.dram_tensor("sinp", [128, T], f32, kind="ExternalInput").ap()
    # mask^T tiles, only used when causal=False
    maskT = nc.dram_tensor("maskT", [S, S], f32, kind="ExternalInput").ap()
    outT = nc.dram_tensor("outT", [D, T], bf16, kind="ExternalOutput").ap()

    hT_b3 = hT.rearrange("(n p) t -> p n t", p=128)     # [128, 16, T]
    wqT_b = wqT.rearrange("(n p) e -> p n e", p=128)
    wkvT_b = wkvT.rearrange("(n p) e -> p n e", p=128)
    woT_b = woT.rearrange("(n p) e -> p n e", p=128)
    outT_b3 = outT.rearrange("(n p) t -> p n t", p=128)  # [128, 16, T]

    Exp = mybir.ActivationFunctionType.Exp
    PSUM = bass.MemorySpace.PSUM

    with tile.TileContext(nc) as tc:
        import contextlib

        with contextlib.ExitStack() as stack:
            const = stack.enter_context(tc.tile_pool(name="const", bufs=1))

            wq_s = const.tile([128, NDB, EQ], bf16)
            wkv_s = const.tile([128, NDB, 2 * HD], bf16)
            wo_s = const.tile([128, 2, D], bf16)
            cos_s = const.tile([128, T], f32)
            sinp_s = const.tile([128, T], f32)
            qt_s = [
                const.tile([128, T], bf16, tag=f"qt{i}", name=f"qt{i}")
                for i in range(2)
            ]
            kt_s = const.tile([128, T], bf16)
            va_s = const.tile([128, T // 128, HD + 1], bf16)
            tri_s = const.tile([128, 4, QT], bf16)
            ident = const.tile([128, 128], f32)

            nc.sync.dma_start(out=wq_s, in_=wqT_b)
            nc.sync.dma_start(out=wkv_s, in_=wkvT_b)
            # cos/sin/wo are not needed until RoPE / step 2 -- load them on
            # the scalar HWDGE ring so they don't delay the first hT
            nc.scalar.dma_start(out=cos_s, in_=cos2)
            nc.scalar.dma_start(out=sinp_s, in_=sinp)
            nc.scalar.dma_start(out=wo_s, in_=woT_b)

            make_identity(nc, ident)
            # ones column of the augmented V
            nc.gpsimd.memset(va_s[:, :, HD : HD + 1], 1.0)
            # multiplicative causal masks for the 4 straddle offsets:
            # tri_s[p, j, f] = 1.0 where f >= p + 128*j else 0.0
            for j in range(4):
                nc.gpsimd.memset(tri_s[:, j, :], 1.0)
                nc.gpsimd.affine_select(
                    out=tri_s[:, j, :],
                    in_=tri_s[:, j, :],
                    compare_op=mybir.AluOpType.is_ge,
                    fill=0.0,
                    base=-128 * j,
                    channel_multiplier=-1,
                    pattern=[[1, QT]],
                )

            # ---------------- pools for the merged pipeline
            with contextlib.ExitStack() as pp:
                ht_pool = pp.enter_context(tc.tile_pool(name="ht", bufs=2))
                # m1 / z / swapped-z rope scratch, all three ropes stacked
                rp_pool = pp.enter_context(tc.tile_pool(name="rp", bufs=1))
                vs_pool = pp.enter_context(tc.tile_pool(name="vs", bufs=2))
                # shared-PSUM pool: proj accumulators, V transposes, Wo tiles
                ps_mm = pp.enter_context(
                    tc.tile_pool(name="ps_mm", bufs=2, space=PSUM)
                )
                ps_s = pp.enter_context(
                    tc.tile_pool(name="ps_s", bufs=2, space=PSUM)
                )
                ps_o = pp.enter_context(
                    tc.tile_pool(name="ps_o", bufs=1, space=PSUM)
                )
                pt_pool = pp.enter_context(tc.tile_pool(name="pt", bufs=4))
                on_pool = pp.enter_context(tc.tile_pool(name="on", bufs=2))
                nm_pool = pp.enter_context(tc.tile_pool(name="nm", bufs=1))
                oa_pool = pp.enter_context(tc.tile_pool(name="oa", bufs=1))
                dr_pool = pp.enter_context(
                    tc.tile_pool(name="dr", bufs=2, space="DRAM")
                )

                def prefetch(it):
                    htile = ht_pool.tile([128, NDB, TT], bf16, tag="ht",
                                         name=f"ht{it}")
                    t0 = it * TT
                    nc.sync.dma_start(out=htile, in_=hT_b3[:, :, t0 : t0 + TT])
                    return htile

                def proj_chunk(it, htile, ri, state):
                    """One projection group (q01 / q23 / kv) + its RoPE muls."""
                    t0 = it * TT
                    tsl = slice(t0, t0 + TT)
                    if ri == 0:
                        state["m1"] = rp_pool.tile([128, 3, TT], f32, tag="m1", name="m1")
                        state["z"] = rp_pool.tile([128, 3, TT], f32, tag="z", name="z")
                        state["m2p"] = rp_pool.tile([128, 3, TT], f32, tag="m2p", name="m2p")
                    m1_all, z_all = state["m1"], state["z"]
                    wsrc, e0, e1, nrows = [
                        (wq_s, 0, 128, 128),
                        (wq_s, 128, 256, 128),
                        (wkv_s, 0, 2 * HD, 64),
                    ][ri]
                    ps = ps_mm.tile([128, TT], f32, tag="mm2k", name=f"pj{ri}")
                    for idb in range(NDB):
                        nc.tensor.matmul(
                            ps,
                            wsrc[:, idb, e0:e1],
                            htile[:, idb, :],
                            start=(idb == 0),
                            stop=(idb == NDB - 1),
                        )
                    # RoPE input products; m2p (swapped z) comes via DMA
                    nc.vector.tensor_mul(
                        m1_all[:nrows, ri, :], ps[:nrows], cos_s[:nrows, tsl]
                    )
                    nc.vector.tensor_mul(
                        z_all[:nrows, ri, :], ps[:nrows], sinp_s[:nrows, tsl]
                    )
                    if ri == 2:
                        state["kv_ps"] = ps

                def proj_tail(it, state):
                    """Swap-DMA + RoPE adds + V transpose for token tile it."""
                    t0 = it * TT
                    tsl = slice(t0, t0 + TT)
                    m1_all, z_all, m2p_all = state["m1"], state["z"], state["m2p"]
                    # partition swap of z (32-row block pairs 0<->1, 2<->3)
                    for c, lo in ((0, 32), (1, 0), (2, 96), (3, 64)):
                        nc.sync.dma_start(
                            out=m2p_all[c * 32 : c * 32 + 32, :, :],
                            in_=z_all[lo : lo + 32, :, :],
                        )
                    # rope adds; k lands twice so odd q-heads can matmul
                    # from partition base 64 (tile_position row packing)
                    nc.vector.tensor_add(
                        kt_s[0:64, tsl], m1_all[0:64, 2, :], m2p_all[0:64, 2, :]
                    )
                    nc.vector.tensor_add(
                        kt_s[64:128, tsl], m1_all[0:64, 2, :], m2p_all[0:64, 2, :]
                    )
                    nc.vector.tensor_add(
                        qt_s[0][:, tsl], m1_all[:, 0, :], m2p_all[:, 0, :]
                    )
                    nc.vector.tensor_add(
                        qt_s[1][:, tsl], m1_all[:, 1, :], m2p_all[:, 1, :]
                    )
                    # V: [d, t] -> [t, d] via PE transpose (V sits at
                    # partitions 64:128 of kv_ps; stage to SBUF first)
                    kv_ps = state["kv_ps"]
                    v_sb = vs_pool.tile([128, TT], f32, tag="v_sb")
                    nc.scalar.copy(out=v_sb[64:128, :], in_=kv_ps[64:128, :])
                    for c4 in range(TT // 128):
                        vt_ps = ps_mm.tile([128, HD], f32, tag="mm2k", name="vt")
                        nc.tensor.transpose(
                            vt_ps,
                            v_sb[64:128, c4 * 128 : (c4 + 1) * 128],
                            ident[64:128, 64:128],
                        )
                        nc.vector.tensor_copy(
                            out=va_s[:, it * 4 + c4, 0:HD], in_=vt_ps
                        )

                def attn_block(it, astate, rp, kb, nkb):
                    """One 128-key attention block of query tile it."""
                    b, iq = it // 4, it % 4
                    q0 = iq * QT
                    bq = b * S + q0
                    qtile = qt_s[rp]
                    if kb == 0:
                        # 2 PSUM banks shared by both head-pairs: rp1's first
                        # A@V slot-waits on rp0's PSUM evacuation copies
                        astate[f"o{rp}"] = [
                            ps_o.tile(
                                [65, QT], f32, tag=f"o{i}", name=f"o{i}", bufs=1
                            )
                            for i in range(2)
                        ]
                    o_ps = astate[f"o{rp}"]
                    ksl = slice(b * S + kb * KBLK, b * S + (kb + 1) * KBLK)
                    j = kb - q0 // KBLK
                    # query-column truncation: straddle block j only
                    # touches queries f >= 128*j
                    c0 = 128 * j if (causal and j > 0) else 0
                    s_ps = ps_s.tile([128, 2, QT], f32, tag="s")
                    pt = pt_pool.tile([128, 2, QT], bf16, tag="pt")
                    for h in range(2):
                        hb = h * 64
                        nc.tensor.matmul(
                            s_ps[:, h, c0:QT],
                            kt_s[hb : hb + 64, ksl],
                            qtile[hb : hb + 64, bq + c0 : bq + QT],
                            start=True,
                            stop=True,
                        )
                    if causal:
                        nc.scalar.activation(
                            pt[:, :, c0:QT], s_ps[:, :, c0:QT], Exp, scale=0.125
                        )
                    else:
                        mk = pt_pool.tile([128, QT], f32, tag="mk")
                        sm = pt_pool.tile([128, 2, QT], f32, tag="sm")
                        nc.sync.dma_start(
                            out=mk,
                            in_=maskT[kb * KBLK : (kb + 1) * KBLK, q0 : q0 + QT],
                        )
                        for h in range(2):
                            nc.vector.scalar_tensor_tensor(
                                out=sm[:, h, :],
                                in0=s_ps[:, h, :],
                                scalar=0.125,
                                in1=mk,
                                op0=mybir.AluOpType.mult,
                                op1=mybir.AluOpType.add,
                            )
                        nc.scalar.activation(pt, sm, Exp, scale=1.0)
                    for h in range(2):
                        if causal and j >= 0:
                            # zero the sub-diagonal triangle in place on the
                            # (otherwise idle) gpsimd: keep where (f-c0) >= p
                            nc.gpsimd.affine_select(
                                out=pt[:, h, c0:QT],
                                in_=pt[:, h, c0:QT],
                                compare_op=mybir.AluOpType.is_ge,
                                fill=0.0,
                                base=0,
                                channel_multiplier=-1,
                                pattern=[[1, QT - c0]],
                            )
                        nc.tensor.matmul(
                            o_ps[h][:, c0:QT],
                            va_s[:, b * (S // 128) + kb, :],
                            pt[:, h, c0:QT],
                            start=(kb == 0),
                            stop=(kb == nkb - 1),
                        )

                def evac_rp(astate, rp):
                    """Evacuate the pair's A@V accumulators (frees the two
                    o PSUM banks for the next head pair)."""
                    ou_all = astate["ou"]
                    o_ps = astate[f"o{rp}"]
                    for h in range(2):
                        nc.vector.tensor_copy(
                            out=ou_all[:, rp * 2 + h, :], in_=o_ps[h]
                        )

                def normalize_tail(astate):
                    """Batched softmax normalization for all 4 heads: the
                    denominator rows bounce through a [32, 64] layout so
                    reciprocal uses 32 lanes, and the partition broadcast is
                    a stride-0 DMA through a DRAM scratch (no engine time)."""
                    on_t, ou_all = astate["on_t"], astate["ou"]
                    r32 = nm_pool.tile([32, 64], f32, tag="r32")
                    nc.sync.dma_start(out=r32, in_=ou_all[64:65, :, :])
                    r32r = nm_pool.tile([32, 64], f32, tag="r32r")
                    nc.vector.reciprocal(r32r, r32)
                    rd = dr_pool.tile([1, 4 * QT], f32, tag="rd", name="rd")
                    nc.sync.dma_start(out=rd, in_=r32r)
                    rec_b = nm_pool.tile([64, 4 * QT], f32, tag="rb")
                    nc.sync.dma_start(
                        out=rec_b, in_=rd.partition_broadcast(64)[:, 0, :]
                    )
                    for rp in range(2):
                        for h in range(2):
                            hh = rp * 2 + h
                            nc.vector.tensor_mul(
                                on_t[rp][h * 64 : h * 64 + 64, :],
                                ou_all[0:64, hh, :],
                                rec_b[:, hh * QT : (hh + 1) * QT],
                            )

                def attn_begin(it):
                    b, iq = it // 4, it % 4
                    nkb = (iq * QT // KBLK + 4) if causal else (S // KBLK)
                    astate = {
                        "on_t": [
                            on_pool.tile(
                                [128, QT], bf16, tag=f"on{i}", name=f"on{i}"
                            )
                            for i in range(2)
                        ],
                        "ou": nm_pool.tile([65, 4, QT], f32, tag="ou", name="ou", bufs=2),
                        "nkb": nkb,
                    }
                    return astate

                def attn_out(it, on_t):
                    """Wo projection + coalesced bf16 output store."""
                    b, iq = it // 4, it % 4
                    q0 = iq * QT
                    qsl = slice(b * S + q0, b * S + q0 + QT)
                    out_acc = oa_pool.tile([128, D // 128, QT], bf16, tag="oacc")
                    for eb in range(D // 128):
                        wo_ps = ps_mm.tile([128, QT], f32, tag="mm2k", name="wo")
                        for db in range(2):
                            nc.tensor.matmul(
                                wo_ps,
                                wo_s[:, db, eb * 128 : (eb + 1) * 128],
                                on_t[db],
                                start=(db == 0),
                                stop=(db == 1),
                            )
                        # split PSUM evacuation: mostly DVE, a bit on ACT
                        if eb % 3 == 1:
                            nc.scalar.copy(out=out_acc[:, eb, :], in_=wo_ps)
                        else:
                            nc.vector.tensor_copy(
                                out=out_acc[:, eb, :], in_=wo_ps
                            )
                    half = D // 256
                    nc.sync.dma_start(
                        out=outT_b3[:, 0:half, qsl], in_=out_acc[:, 0:half, :]
                    )
                    nc.sync.dma_start(
                        out=outT_b3[:, half:, qsl], in_=out_acc[:, half:, :]
                    )

                # ---------------- the software-pipelined merged loop:
                # attention for tile s-1, projection for tile s, Wo for tile
                # s-2.  The 2-step Wo skew means the Wo matmuls' inputs are
                # always long-ready (they fill PE gaps, and their PSUM
                # evacuation copies never head-of-line-block the exps), and
                # the normalize chain of s-1 has a full step to complete.
                htiles = {0: prefetch(0)}
                on_hist = {}
                for step in range(NT + 2):
                    if step + 1 <= NT - 1:
                        htiles[step + 1] = prefetch(step + 1)
                    if 1 <= step <= NT:
                        astate = attn_begin(step - 1)
                        nkb = astate["nkb"]
                        for rp in range(2):
                            for kb in range(nkb):
                                attn_block(step - 1, astate, rp, kb, nkb)
                            evac_rp(astate, rp)
                        normalize_tail(astate)
                        on_hist[step - 1] = astate["on_t"]
                    if step <= NT - 1:
                        pstate = {}
                        htile = htiles.pop(step)
                        for ri in range(3):
                            proj_chunk(step, htile, ri, pstate)
                        proj_tail(step, pstate)
                    if step >= 2:
                        attn_out(step - 2, on_hist.pop(step - 2))

    nc.compile()
    return nc


def _host_inputs(inputs, causal):
    """Shard + transpose the full inputs into 8 per-core input maps."""
    h = np.asarray(inputs["hidden_states"], np.float32)
    cos = np.asarray(inputs["position_cos"], np.float32)
    sin = np.asarray(inputs["position_sin"], np.float32)
    Wq = np.asarray(inputs["Wq"], np.float32)
    Wk = np.asarray(inputs["Wk"], np.float32)
    Wv = np.asarray(inputs["Wv"], np.float32)
    Wo = np.asarray(inputs["Wo"], np.float32)
    mask = np.asarray(inputs["attention_mask"], np.float32)[0, 0]

    hT = np.ascontiguousarray(h.reshape(T, D).T).astype(BF16)

    cosT = np.tile(cos.T, (1, B))                     # [64, T]
    sinT = np.tile(sin.T, (1, B))
    cos2 = np.ascontiguousarray(np.vstack([cosT, cosT]).astype(np.float32))
    s_signed = np.vstack([-sinT[0:32], sinT[32:64]])  # rot_half sign baked in
    sin2s = np.vstack([s_signed, s_signed])           # [128, T]
    # pre-swap so that z[p] = x[p]*sinp[p]; m2[p] = z[swap(p)] equals
    # rot_half(x)[p] * sin_signed[p]  (swap = 32-row block pairs 0<->1, 2<->3)
    swap_idx = np.concatenate(
        [np.arange(32, 64), np.arange(0, 32), np.arange(96, 128), np.arange(64, 96)]
    )
    sinp = np.ascontiguousarray(sin2s[swap_idx].astype(np.float32))

    maskT = np.ascontiguousarray(mask.T).astype(np.float32)

    in_maps = []
    for g in range(8):
        in_maps.append(
            {
                "hT": hT,
                "wqT": np.ascontiguousarray(
                    Wq[g * EQ : (g + 1) * EQ].T
                ).astype(BF16),
                "wkvT": np.ascontiguousarray(
                    np.concatenate(
                        [
                            Wk[g * HD : (g + 1) * HD].T,
                            Wv[g * HD : (g + 1) * HD].T,
                        ],
                        axis=1,
                    )
                ).astype(BF16),
                "woT": np.ascontiguousarray(
                    Wo[:, g * EQ : (g + 1) * EQ].T
                ).astype(BF16),
                "cos2": cos2,
                "sinp": sinp,
                "maskT": maskT,
            }
        )
    return in_maps


def _is_causal(mask):
    m = np.asarray(mask, np.float32)[0, 0]
    tri = np.tril(np.ones((S, S), bool))
    return bool(np.all(m[tri] == 0.0) and np.all(m[~tri] <= -1e8))


def _assemble(results):
    acc = np.zeros((D, T), np.float32)
    for r in results:
        acc += r["outT"].astype(np.float32)
    return np.ascontiguousarray(acc.reshape(D, B, S).transpose(1, 2, 0))


def kernel(**inputs) -> np.ndarray:
    from concourse.bass_utils import run_bass_kernel_spmd

    causal = _is_causal(inputs["attention_mask"])
    key = ("prog", causal)
    if key not in _CACHE:
        _CACHE[key] = _build_program(causal)
    nc = _CACHE[key]

    in_maps = _host_inputs(inputs, causal)
    res = run_bass_kernel_spmd(nc, in_maps, core_ids=list(range(8)))
    return _assemble(res.results)


# revision 34
# speedup vs baseline: 1.1495x; 1.0027x over previous
"""Trainium2 Bass kernel for GQA multi-head attention (nn_MultiHeadAttention).

Reference computation (fp32):
    q = h @ Wq^T -> RoPE ; k = h @ Wk^T -> RoPE ; v = h @ Wv^T
    scores = q k^T / sqrt(64) + causal_mask ; w = softmax(scores)
    out = (w v) @ Wo^T

Shapes: h [2,2048,2048], Wq [2048,2048], Wk/Wv [512,2048], Wo [2048,2048],
32 q heads / 8 kv heads (GQA group=4), head_dim 64.

Sharding: tensor-parallel over the 8 kv-head groups, one group per core.
Core g owns q heads [4g,4g+4), kv head g, Wo columns [256g, 256(g+1)).
Each core computes a full-token partial of the output projection; the host
sums the 8 partials (the Wo contraction splits over head blocks).

Structure: a single software-pipelined loop over the 8 token tiles of 512;
at step s it emits [hT prefetch for s+1] [attention for query tile s-1]
[QKV projection + RoPE for tile s] [Wo projection + output store for tile
s-2].  The 2-step Wo skew means the Wo matmuls' inputs are always
long-ready (they fill PE gaps so the HAM clock gate stays warm, and their
PSUM-evacuation copies never head-of-line-block the exps), and the softmax
normalize chain of tile s-1 gets a full step of slack before anything
consumes it.  Engine assignment keeps each FIFO's latency-critical ops
away from bulk work: PE matmuls; ACT exps + a slice of the PSUM
evacuations; DVE RoPE muls/adds + evacuation copies + reciprocal; GPSIMD
causal masks only; the normalize partition-broadcast is a stride-0 DMA
through a DRAM scratch (no compute engine involved).

Everything is kept transposed: h^T [2048, 4096] comes in, Q^T/K^T [d, t]
fall out of the projections directly, scores are S^T[k, q], softmax is a
plain exp (scores are O(5), fp32-safe) with causal-skip at 128-key-block
granularity AND 128-query-column truncation inside the diagonal straddle
blocks (scores / exp / mask / A@V all skip the dead triangle).  A@V uses V
augmented with a ones-column so softmax denominators fall out of the same
matmul.  RoPE's rot_half partition swap runs on a pre-scaled copy
(z = x * sin_pre, then swap z via SBUF DMA) so no scalar-engine staging
copy is needed; the odd-head K replica is a second (partition-shifted)
DVE add instead of a DMA.  Denominator reciprocals for all 4 heads of a
query tile are batched through one [32, 64] bounce so the DVE reciprocal
uses 32 lanes.  Output partials are stored bf16 (halves the HBM write) as
one coalesced DMA per query tile.
"""

import sys

for _p in ("/opt/trn_rl_repo",):
    if _p not in sys.path:
        sys.path.insert(0, _p)

import numpy as np
import ml_dtypes

D = 2048          # model dim
HD = 64           # head dim
S = 2048          # sequence
B = 2             # batch
T = B * S         # total tokens
EQ = 256          # q-projection rows per core (4 heads x 64)
TT = 512          # token tile (both projection and query tile)
NT = T // TT      # 8 merged steps
NDB = D // 128    # contraction blocks for projections
QT = 512          # query tile for attention
KBLK = 128        # key block for attention
BF16 = ml_dtypes.bfloat16

_CACHE = {}


def _build_program(causal: bool):
    """Build the single-core Bass/Tile program (identical across cores)."""
    import concourse.bass as bass
    import concourse.mybir as mybir
    import concourse.tile as tile
    from concourse import bacc
    from concourse.masks import make_identity

    f32 = mybir.dt.float32
    bf16 = mybir.dt.bfloat16
    f8 = mybir.dt.float8e4

    nc = bacc.Bacc("TRN2", target_bir_lowering=False, debug=False)

    hT = nc.dram_tensor("hT", [D, T], bf16, kind="ExternalInput").ap()
    wqT = nc.dram_tensor("wqT", [D, EQ], bf16, kind="ExternalInput").ap()
    # k and v projection weights packed [D, 64+64] so one matmul produces both
    wkvT = nc.dram_tensor("wkvT", [D, 2 * HD], bf16, kind="ExternalInput").ap()
    woT = nc.dram_tensor("woT", [EQ, D], bf16, kind="ExternalInput").ap()
    cos2 = nc.dram_tensor("cos2", [128, T], f32, kind="ExternalInput").ap()
    # sin with rot_half sign AND partition swap pre-applied (see _host_inputs)
    sinp = nc.dram_tensor("sinp", [128, T], f32, kind="ExternalInput").ap()
    # mask^T tiles, only used when causal=False
    maskT = nc.dram_tensor("maskT", [S, S], f32, kind="ExternalInput").ap()
    outT = nc.dram_tensor("outT", [D, T], bf16, kind="ExternalOutput").ap()

    hT_b3 = hT.rearrange("(n p) t -> p n t", p=128)     # [128, 16, T]
    wqT_b = wqT.rearrange("(n p) e -> p n e", p=128)
    wkvT_b = wkvT.rearrange("(n p) e -> p n e", p=128)
    woT_b = woT.rearrange("(n p) e -> p n e", p=128)
    outT_b3 = outT.rearrange("(n p) t -> p n t", p=128)  # [128, 16, T]

    Exp = mybir.ActivationFunctionType.Exp
    PSUM = bass.MemorySpace.PSUM

    with tile.TileContext(nc) as tc:
        import contextlib

        with contextlib.ExitStack() as stack:
            const = stack.enter_context(tc.tile_pool(name="const", bufs=1))

            wq_s = const.tile([128, NDB, EQ], bf16)
            wkv_s = const.tile([128, NDB, 2 * HD], bf16)
            wo_s = const.tile([128, 2, D], bf16)
            cos_s = const.tile([128, T], f32)
            sinp_s = const.tile([128, T], f32)
            qt_s = [
                const.tile([128, T], bf16, tag=f"qt{i}", name=f"qt{i}")
                for i in range(2)
            ]
            kt_s = const.tile([128, T], bf16)
            va_s = const.tile([128, T // 128, HD + 1], bf16)
            tri_s = const.tile([128, 4, QT], bf16)
            ident = const.tile([128, 128], f32)

            nc.sync.dma_start(out=wq_s, in_=wqT_b)
            nc.sync.dma_start(out=wkv_s, in_=wkvT_b)
            nc.sync.dma_start(out=cos_s, in_=cos2)
            nc.sync.dma_start(out=sinp_s, in_=sinp)
            # wo is not needed until step 2 -- load it on the scalar HWDGE
            # ring so it doesn't delay the first hT prefetch
            nc.scalar.dma_start(out=wo_s, in_=woT_b)

            make_identity(nc, ident)
            # ones column of the augmented V
            nc.gpsimd.memset(va_s[:, :, HD : HD + 1], 1.0)
            # multiplicative causal masks for the 4 straddle offsets:
            # tri_s[p, j, f] = 1.0 where f >= p + 128*j else 0.0
            for j in range(4):
                nc.gpsimd.memset(tri_s[:, j, :], 1.0)
                nc.gpsimd.affine_select(
                    out=tri_s[:, j, :],
                    in_=tri_s[:, j, :],
                    compare_op=mybir.AluOpType.is_ge,
                    fill=0.0,
                    base=-128 * j,
                    channel_multiplier=-1,
                    pattern=[[1, QT]],
                )

            # ---------------- pools for the merged pipeline
            with contextlib.ExitStack() as pp:
                ht_pool = pp.enter_context(tc.tile_pool(name="ht", bufs=2))
                # m1 / z / swapped-z rope scratch, all three ropes stacked
                rp_pool = pp.enter_context(tc.tile_pool(name="rp", bufs=1))
                vs_pool = pp.enter_context(tc.tile_pool(name="vs", bufs=2))
                # shared-PSUM pool: proj accumulators, V transposes, Wo tiles
                ps_mm = pp.enter_context(
                    tc.tile_pool(name="ps_mm", bufs=2, space=PSUM)
                )
                ps_s = pp.enter_context(
                    tc.tile_pool(name="ps_s", bufs=2, space=PSUM)
                )
                ps_o = pp.enter_context(
                    tc.tile_pool(name="ps_o", bufs=1, space=PSUM)
                )
                pt_pool = pp.enter_context(tc.tile_pool(name="pt", bufs=4))
                on_pool = pp.enter_context(tc.tile_pool(name="on", bufs=2))
                nm_pool = pp.enter_context(tc.tile_pool(name="nm", bufs=1))
                oa_pool = pp.enter_context(tc.tile_pool(name="oa", bufs=1))
                dr_pool = pp.enter_context(
                    tc.tile_pool(name="dr", bufs=2, space="DRAM")
                )

                def prefetch(it):
                    htile = ht_pool.tile([128, NDB, TT], bf16, tag="ht",
                                         name=f"ht{it}")
                    t0 = it * TT
                    nc.sync.dma_start(out=htile, in_=hT_b3[:, :, t0 : t0 + TT])
                    return htile

                def proj_chunk(it, htile, ri, state):
                    """One projection group (q01 / q23 / kv) + its RoPE muls."""
                    t0 = it * TT
                    tsl = slice(t0, t0 + TT)
                    if ri == 0:
                        state["m1"] = rp_pool.tile([128, 3, TT], f32, tag="m1", name="m1")
                        state["z"] = rp_pool.tile([128, 3, TT], f32, tag="z", name="z")
                        state["m2p"] = rp_pool.tile([128, 3, TT], f32, tag="m2p", name="m2p")
                    m1_all, z_all = state["m1"], state["z"]
                    wsrc, e0, e1, nrows = [
                        (wq_s, 0, 128, 128),
                        (wq_s, 128, 256, 128),
                        (wkv_s, 0, 2 * HD, 64),
                    ][ri]
                    ps = ps_mm.tile([128, TT], f32, tag="mm2k", name=f"pj{ri}")
                    for idb in range(NDB):
                        nc.tensor.matmul(
                            ps,
                            wsrc[:, idb, e0:e1],
                            htile[:, idb, :],
                            start=(idb == 0),
                            stop=(idb == NDB - 1),
                        )
                    # RoPE input products; m2p (swapped z) comes via DMA
                    nc.vector.tensor_mul(
                        m1_all[:nrows, ri, :], ps[:nrows], cos_s[:nrows, tsl]
                    )
                    nc.vector.tensor_mul(
                        z_all[:nrows, ri, :], ps[:nrows], sinp_s[:nrows, tsl]
                    )
                    if ri == 2:
                        state["kv_ps"] = ps

                def proj_tail(it, state):
                    """Swap-DMA + RoPE adds + V transpose for token tile it."""
                    t0 = it * TT
                    tsl = slice(t0, t0 + TT)
                    m1_all, z_all, m2p_all = state["m1"], state["z"], state["m2p"]
                    # partition swap of z (32-row block pairs 0<->1, 2<->3)
                    for c, lo in ((0, 32), (1, 0), (2, 96), (3, 64)):
                        nc.sync.dma_start(
                            out=m2p_all[c * 32 : c * 32 + 32, :, :],
                            in_=z_all[lo : lo + 32, :, :],
                        )
                    # rope adds; k lands twice so odd q-heads can matmul
                    # from partition base 64 (tile_position row packing)
                    nc.vector.tensor_add(
                        kt_s[0:64, tsl], m1_all[0:64, 2, :], m2p_all[0:64, 2, :]
                    )
                    nc.vector.tensor_add(
                        kt_s[64:128, tsl], m1_all[0:64, 2, :], m2p_all[0:64, 2, :]
                    )
                    nc.vector.tensor_add(
                        qt_s[0][:, tsl], m1_all[:, 0, :], m2p_all[:, 0, :]
                    )
                    nc.vector.tensor_add(
                        qt_s[1][:, tsl], m1_all[:, 1, :], m2p_all[:, 1, :]
                    )
                    # V: [d, t] -> [t, d] via PE transpose (V sits at
                    # partitions 64:128 of kv_ps; stage to SBUF first)
                    kv_ps = state["kv_ps"]
                    v_sb = vs_pool.tile([128, TT], f32, tag="v_sb")
                    nc.scalar.copy(out=v_sb[64:128, :], in_=kv_ps[64:128, :])
                    for c4 in range(TT // 128):
                        vt_ps = ps_mm.tile([128, HD], f32, tag="mm2k", name="vt")
                        nc.tensor.transpose(
                            vt_ps,
                            v_sb[64:128, c4 * 128 : (c4 + 1) * 128],
                            ident[64:128, 64:128],
                        )
                        nc.vector.tensor_copy(
                            out=va_s[:, it * 4 + c4, 0:HD], in_=vt_ps
                        )

                def attn_block(it, astate, rp, kb, nkb):
                    """One 128-key attention block of query tile it."""
                    b, iq = it // 4, it % 4
                    q0 = iq * QT
                    bq = b * S + q0
                    qtile = qt_s[rp]
                    if kb == 0:
                        # 2 PSUM banks shared by both head-pairs: rp1's first
                        # A@V slot-waits on rp0's PSUM evacuation copies
                        astate[f"o{rp}"] = [
                            ps_o.tile(
                                [65, QT], f32, tag=f"o{i}", name=f"o{i}", bufs=1
                            )
                            for i in range(2)
                        ]
                    o_ps = astate[f"o{rp}"]
                    ksl = slice(b * S + kb * KBLK, b * S + (kb + 1) * KBLK)
                    j = kb - q0 // KBLK
                    # query-column truncation: straddle block j only
                    # touches queries f >= 128*j
                    c0 = 128 * j if (causal and j > 0) else 0
                    s_ps = ps_s.tile([128, 2, QT], f32, tag="s")
                    pt = pt_pool.tile([128, 2, QT], bf16, tag="pt")
                    for h in range(2):
                        hb = h * 64
                        nc.tensor.matmul(
                            s_ps[:, h, c0:QT],
                            kt_s[hb : hb + 64, ksl],
                            qtile[hb : hb + 64, bq + c0 : bq + QT],
                            start=True,
                            stop=True,
                        )
                    if causal:
                        nc.scalar.activation(
                            pt[:, :, c0:QT], s_ps[:, :, c0:QT], Exp, scale=0.125
                        )
                    else:
                        mk = pt_pool.tile([128, QT], f32, tag="mk")
                        sm = pt_pool.tile([128, 2, QT], f32, tag="sm")
                        nc.sync.dma_start(
                            out=mk,
                            in_=maskT[kb * KBLK : (kb + 1) * KBLK, q0 : q0 + QT],
                        )
                        for h in range(2):
                            nc.vector.scalar_tensor_tensor(
                                out=sm[:, h, :],
                                in0=s_ps[:, h, :],
                                scalar=0.125,
                                in1=mk,
                                op0=mybir.AluOpType.mult,
                                op1=mybir.AluOpType.add,
                            )
                        nc.scalar.activation(pt, sm, Exp, scale=1.0)
                    for h in range(2):
                        if causal and j >= 0:
                            # zero the sub-diagonal triangle in place on the
                            # (otherwise idle) gpsimd: keep where (f-c0) >= p
                            nc.gpsimd.affine_select(
                                out=pt[:, h, c0:QT],
                                in_=pt[:, h, c0:QT],
                                compare_op=mybir.AluOpType.is_ge,
                                fill=0.0,
                                base=0,
                                channel_multiplier=-1,
                                pattern=[[1, QT - c0]],
                            )
                        nc.tensor.matmul(
                            o_ps[h][:, c0:QT],
                            va_s[:, b * (S // 128) + kb, :],
                            pt[:, h, c0:QT],
                            start=(kb == 0),
                            stop=(kb == nkb - 1),
                        )

                def evac_rp(astate, rp):
                    """Evacuate the pair's A@V accumulators (frees the two
                    o PSUM banks for the next head pair)."""
                    ou_all = astate["ou"]
                    o_ps = astate[f"o{rp}"]
                    for h in range(2):
                        nc.vector.tensor_copy(
                            out=ou_all[:, rp * 2 + h, :], in_=o_ps[h]
                        )

                def normalize_tail(astate):
                    """Batched softmax normalization for all 4 heads: the
                    denominator rows bounce through a [32, 64] layout so
                    reciprocal uses 32 lanes, and the partition broadcast is
                    a stride-0 DMA through a DRAM scratch (no engine time)."""
                    on_t, ou_all = astate["on_t"], astate["ou"]
                    r32 = nm_pool.tile([32, 64], f32, tag="r32")
                    nc.sync.dma_start(out=r32, in_=ou_all[64:65, :, :])
                    r32r = nm_pool.tile([32, 64], f32, tag="r32r")
                    nc.vector.reciprocal(r32r, r32)
                    rd = dr_pool.tile([1, 4 * QT], f32, tag="rd", name="rd")
                    nc.sync.dma_start(out=rd, in_=r32r)
                    rec_b = nm_pool.tile([64, 4 * QT], f32, tag="rb")
                    nc.sync.dma_start(
                        out=rec_b, in_=rd.partition_broadcast(64)[:, 0, :]
                    )
                    for rp in range(2):
                        for h in range(2):
                            hh = rp * 2 + h
                            nc.vector.tensor_mul(
                                on_t[rp][h * 64 : h * 64 + 64, :],
                                ou_all[0:64, hh, :],
                                rec_b[:, hh * QT : (hh + 1) * QT],
                            )

                def attn_begin(it):
                    b, iq = it // 4, it % 4
                    nkb = (iq * QT // KBLK + 4) if causal else (S // KBLK)
                    astate = {
                        "on_t": [
                            on_pool.tile(
                                [128, QT], bf16, tag=f"on{i}", name=f"on{i}"
                            )
                            for i in range(2)
                        ],
                        "ou": nm_pool.tile([65, 4, QT], f32, tag="ou", name="ou", bufs=2),
                        "nkb": nkb,
                    }
                    return astate

                def attn_out(it, on_t):
                    """Wo projection + coalesced bf16 output store."""
                    b, iq = it // 4, it % 4
                    q0 = iq * QT
                    qsl = slice(b * S + q0, b * S + q0 + QT)
                    out_acc = oa_pool.tile([128, D // 128, QT], bf16, tag="oacc")
                    for eb in range(D // 128):
                        wo_ps = ps_mm.tile([128, QT], f32, tag="mm2k", name="wo")
                        for db in range(2):
                            nc.tensor.matmul(
                                wo_ps,
                                wo_s[:, db, eb * 128 : (eb + 1) * 128],
                                on_t[db],
                                start=(db == 0),
                                stop=(db == 1),
                            )
                        # split PSUM evacuation: mostly DVE, a bit on ACT
                        if eb % 3 == 1:
                            nc.scalar.copy(out=out_acc[:, eb, :], in_=wo_ps)
                        else:
                            nc.vector.tensor_copy(
                                out=out_acc[:, eb, :], in_=wo_ps
                            )
                    half = D // 256
                    nc.sync.dma_start(
                        out=outT_b3[:, 0:half, qsl], in_=out_acc[:, 0:half, :]
                    )
                    nc.sync.dma_start(
                        out=outT_b3[:, half:, qsl], in_=out_acc[:, half:, :]
                    )

                # ---------------- the software-pipelined merged loop:
                # attention for tile s-1, projection for tile s, Wo for tile
                # s-2.  The 2-step Wo skew means the Wo matmuls' inputs are
                # always long-ready (they fill PE gaps, and their PSUM
                # evacuation copies never head-of-line-block the exps), and
                # the normalize chain of s-1 has a full step to complete.
                htiles = {0: prefetch(0)}
                on_hist = {}
                for step in range(NT + 2):
                    if step + 1 <= NT - 1:
                        htiles[step + 1] = prefetch(step + 1)
                    if 1 <= step <= NT:
                        astate = attn_begin(step - 1)
                        nkb = astate["nkb"]
                        for rp in range(2):
                            for kb in range(nkb):
                                attn_block(step - 1, astate, rp, kb, nkb)
                            evac_rp(astate, rp)
                        normalize_tail(astate)
                        on_hist[step - 1] = astate["on_t"]
                    if step <= NT - 1:
                        pstate = {}
                        htile = htiles.pop(step)
                        for ri in range(3):
                            proj_chunk(step, htile, ri, pstate)
                        proj_tail(step, pstate)
                    if step >= 2:
                        attn_out(step - 2, on_hist.pop(step - 2))

    nc.compile()
    return nc


def _host_inputs(inputs, causal):
    """Shard + transpose the full inputs into 8 per-core input maps."""
    h = np.asarray(inputs["hidden_states"], np.float32)
    cos = np.asarray(inputs["position_cos"], np.float32)
    sin = np.asarray(inputs["position_sin"], np.float32)
    Wq = np.asarray(inputs["Wq"], np.float32)
    Wk = np.asarray(inputs["Wk"], np.float32)
    Wv = np.asarray(inputs["Wv"], np.float32)
    Wo = np.asarray(inputs["Wo"], np.float32)
    mask = np.asarray(inputs["attention_mask"], np.float32)[0, 0]

    hT = np.ascontiguousarray(h.reshape(T, D).T).astype(BF16)

    cosT = np.tile(cos.T, (1, B))                     # [64, T]
    sinT = np.tile(sin.T, (1, B))
    cos2 = np.ascontiguousarray(np.vstack([cosT, cosT]).astype(np.float32))
    s_signed = np.vstack([-sinT[0:32], sinT[32:64]])  # rot_half sign baked in
    sin2s = np.vstack([s_signed, s_signed])           # [128, T]
    # pre-swap so that z[p] = x[p]*sinp[p]; m2[p] = z[swap(p)] equals
    # rot_half(x)[p] * sin_signed[p]  (swap = 32-row block pairs 0<->1, 2<->3)
    swap_idx = np.concatenate(
        [np.arange(32, 64), np.arange(0, 32), np.arange(96, 128), np.arange(64, 96)]
    )
    sinp = np.ascontiguousarray(sin2s[swap_idx].astype(np.float32))

    maskT = np.ascontiguousarray(mask.T).astype(np.float32)

    in_maps = []
    for g in range(8):
        in_maps.append(
            {
                "hT": hT,
                "wqT": np.ascontiguousarray(
                    Wq[g * EQ : (g + 1) * EQ].T
                ).astype(BF16),
                "wkvT": np.ascontiguousarray(
                    np.concatenate(
                        [
                            Wk[g * HD : (g + 1) * HD].T,
                            Wv[g * HD : (g + 1) * HD].T,
                        ],
                        axis=1,
                    )
                ).astype(BF16),
                "woT": np.ascontiguousarray(
                    Wo[:, g * EQ : (g + 1) * EQ].T
                ).astype(BF16),
                "cos2": cos2,
                "sinp": sinp,
                "maskT": maskT,
            }
        )
    return in_maps


def _is_causal(mask):
    m = np.asarray(mask, np.float32)[0, 0]
    tri = np.tril(np.ones((S, S), bool))
    return bool(np.all(m[tri] == 0.0) and np.all(m[~tri] <= -1e8))


def _assemble(results):
    acc = np.zeros((D, T), np.float32)
    for r in results:
        acc += r["outT"].astype(np.float32)
    return np.ascontiguousarray(acc.reshape(D, B, S).transpose(1, 2, 0))


def kernel(**inputs) -> np.ndarray:
    from concourse.bass_utils import run_bass_kernel_spmd

    causal = _is_causal(inputs["attention_mask"])
    key = ("prog", causal)
    if key not in _CACHE:
        _CACHE[key] = _build_program(causal)
    nc = _CACHE[key]

    in_maps = _host_inputs(inputs, causal)
    res = run_bass_kernel_spmd(nc, in_maps, core_ids=list(range(8)))
    return _assemble(res.results)


# revision 35
# speedup vs baseline: 1.1662x; 1.0145x over previous
"""Trainium2 Bass kernel for GQA multi-head attention (nn_MultiHeadAttention).

Reference computation (fp32):
    q = h @ Wq^T -> RoPE ; k = h @ Wk^T -> RoPE ; v = h @ Wv^T
    scores = q k^T / sqrt(64) + causal_mask ; w = softmax(scores)
    out = (w v) @ Wo^T

Shapes: h [2,2048,2048], Wq [2048,2048], Wk/Wv [512,2048], Wo [2048,2048],
32 q heads / 8 kv heads (GQA group=4), head_dim 64.

Sharding: tensor-parallel over the 8 kv-head groups, one group per core.
Core g owns q heads [4g,4g+4), kv head g, Wo columns [256g, 256(g+1)).
Each core computes a full-token partial of the output projection; the host
sums the 8 partials (the Wo contraction splits over head blocks).

Structure: a single software-pipelined loop over the 8 token tiles of 512;
at step s it emits [hT prefetch for s+1] [attention for query tile s-1]
[QKV projection + RoPE for tile s] [Wo projection + output store for tile
s-2].  The 2-step Wo skew means the Wo matmuls' inputs are always
long-ready (they fill PE gaps so the HAM clock gate stays warm, and their
PSUM-evacuation copies never head-of-line-block the exps), and the softmax
normalize chain of tile s-1 gets a full step of slack before anything
consumes it.  Engine assignment keeps each FIFO's latency-critical ops
away from bulk work: PE matmuls; ACT exps + a slice of the PSUM
evacuations; DVE RoPE muls/adds + evacuation copies + reciprocal; GPSIMD
causal masks only; the normalize partition-broadcast is a stride-0 DMA
through a DRAM scratch (no compute engine involved).

Everything is kept transposed: h^T [2048, 4096] comes in, Q^T/K^T [d, t]
fall out of the projections directly, scores are S^T[k, q], softmax is a
plain exp (scores are O(5), fp32-safe) with causal-skip at 128-key-block
granularity AND 128-query-column truncation inside the diagonal straddle
blocks (scores / exp / mask / A@V all skip the dead triangle).  A@V uses V
augmented with a ones-column so softmax denominators fall out of the same
matmul.  RoPE's rot_half partition swap runs on a pre-scaled copy
(z = x * sin_pre, then swap z via SBUF DMA) so no scalar-engine staging
copy is needed; the odd-head K replica is a second (partition-shifted)
DVE add instead of a DMA.  Denominator reciprocals for all 4 heads of a
query tile are batched through one [32, 64] bounce so the DVE reciprocal
uses 32 lanes.  Output partials are stored bf16 (halves the HBM write) as
one coalesced DMA per query tile.
"""

import sys

for _p in ("/opt/trn_rl_repo",):
    if _p not in sys.path:
        sys.path.insert(0, _p)

import numpy as np
import ml_dtypes

D = 2048          # model dim
HD = 64           # head dim
S = 2048          # sequence
B = 2             # batch
T = B * S         # total tokens
EQ = 256          # q-projection rows per core (4 heads x 64)
TT = 512          # token tile (both projection and query tile)
NT = T // TT      # 8 merged steps
NDB = D // 128    # contraction blocks for projections
QT = 512          # query tile for attention
KBLK = 128        # key block for attention
BF16 = ml_dtypes.bfloat16

_CACHE = {}


def _build_program(causal: bool):
    """Build the single-core Bass/Tile program (identical across cores)."""
    import concourse.bass as bass
    import concourse.mybir as mybir
    import concourse.tile as tile
    from concourse import bacc
    from concourse.masks import make_identity

    f32 = mybir.dt.float32
    bf16 = mybir.dt.bfloat16
    f8 = mybir.dt.float8e4

    nc = bacc.Bacc("TRN2", target_bir_lowering=False, debug=False)

    hT = nc.dram_tensor("hT", [D, T], bf16, kind="ExternalInput").ap()
    wqT = nc.dram_tensor("wqT", [D, EQ], bf16, kind="ExternalInput").ap()
    # k and v projection weights packed [D, 64+64] so one matmul produces both
    wkvT = nc.dram_tensor("wkvT", [D, 2 * HD], bf16, kind="ExternalInput").ap()
    woT = nc.dram_tensor("woT", [EQ, D], bf16, kind="ExternalInput").ap()
    cos2 = nc.dram_tensor("cos2", [128, T], f32, kind="ExternalInput").ap()
    # sin with rot_half sign AND partition swap pre-applied (see _host_inputs)
    sinp = nc.dram_tensor("sinp", [128, T], f32, kind="ExternalInput").ap()
    # mask^T tiles, only used when causal=False
    maskT = nc.dram_tensor("maskT", [S, S], f32, kind="ExternalInput").ap()
    outT = nc.dram_tensor("outT", [D, T], bf16, kind="ExternalOutput").ap()

    hT_b3 = hT.rearrange("(n p) t -> p n t", p=128)     # [128, 16, T]
    wqT_b = wqT.rearrange("(n p) e -> p n e", p=128)
    wkvT_b = wkvT.rearrange("(n p) e -> p n e", p=128)
    woT_b = woT.rearrange("(n p) e -> p n e", p=128)
    outT_b3 = outT.rearrange("(n p) t -> p n t", p=128)  # [128, 16, T]

    Exp = mybir.ActivationFunctionType.Exp
    PSUM = bass.MemorySpace.PSUM

    with tile.TileContext(nc) as tc:
        import contextlib

        with contextlib.ExitStack() as stack:
            const = stack.enter_context(tc.tile_pool(name="const", bufs=1))

            wq_s = const.tile([128, NDB, EQ], bf16)
            wkv_s = const.tile([128, NDB, 2 * HD], bf16)
            wo_s = const.tile([128, 2, D], bf16)
            cos_s = const.tile([128, T], f32)
            sinp_s = const.tile([128, T], f32)
            qt_s = [
                const.tile([128, T], bf16, tag=f"qt{i}", name=f"qt{i}")
                for i in range(2)
            ]
            kt_s = const.tile([128, T], bf16)
            va_s = const.tile([128, T // 128, HD + 1], bf16)
            tri_s = const.tile([128, 4, QT], bf16)
            ident = const.tile([128, 128], f32)

            nc.sync.dma_start(out=wq_s, in_=wqT_b)
            nc.sync.dma_start(out=wkv_s, in_=wkvT_b)
            nc.sync.dma_start(out=cos_s, in_=cos2)
            nc.sync.dma_start(out=sinp_s, in_=sinp)
            # wo is not needed until step 2 -- load it on the scalar HWDGE
            # ring so it doesn't delay the first hT prefetch
            nc.scalar.dma_start(out=wo_s, in_=woT_b)

            make_identity(nc, ident)
            # ones column of the augmented V
            nc.gpsimd.memset(va_s[:, :, HD : HD + 1], 1.0)
            # multiplicative causal masks for the 4 straddle offsets:
            # tri_s[p, j, f] = 1.0 where f >= p + 128*j else 0.0
            for j in range(4):
                nc.gpsimd.memset(tri_s[:, j, :], 1.0)
                nc.gpsimd.affine_select(
                    out=tri_s[:, j, :],
                    in_=tri_s[:, j, :],
                    compare_op=mybir.AluOpType.is_ge,
                    fill=0.0,
                    base=-128 * j,
                    channel_multiplier=-1,
                    pattern=[[1, QT]],
                )

            # ---------------- pools for the merged pipeline
            with contextlib.ExitStack() as pp:
                ht_pool = pp.enter_context(tc.tile_pool(name="ht", bufs=3))
                # m1 / z / swapped-z rope scratch, all three ropes stacked
                rp_pool = pp.enter_context(tc.tile_pool(name="rp", bufs=1))
                vs_pool = pp.enter_context(tc.tile_pool(name="vs", bufs=2))
                # shared-PSUM pool: proj accumulators, V transposes, Wo tiles
                ps_mm = pp.enter_context(
                    tc.tile_pool(name="ps_mm", bufs=2, space=PSUM)
                )
                ps_s = pp.enter_context(
                    tc.tile_pool(name="ps_s", bufs=2, space=PSUM)
                )
                ps_o = pp.enter_context(
                    tc.tile_pool(name="ps_o", bufs=1, space=PSUM)
                )
                pt_pool = pp.enter_context(tc.tile_pool(name="pt", bufs=4))
                on_pool = pp.enter_context(tc.tile_pool(name="on", bufs=2))
                nm_pool = pp.enter_context(tc.tile_pool(name="nm", bufs=1))
                oa_pool = pp.enter_context(tc.tile_pool(name="oa", bufs=1))
                dr_pool = pp.enter_context(
                    tc.tile_pool(name="dr", bufs=2, space="DRAM")
                )

                def prefetch(it):
                    htile = ht_pool.tile([128, NDB, TT], bf16, tag="ht",
                                         name=f"ht{it}")
                    t0 = it * TT
                    nc.sync.dma_start(out=htile, in_=hT_b3[:, :, t0 : t0 + TT])
                    return htile

                def proj_chunk(it, htile, ri, state):
                    """One projection group (q01 / q23 / kv) + its RoPE muls."""
                    t0 = it * TT
                    tsl = slice(t0, t0 + TT)
                    if ri == 0:
                        state["m1"] = rp_pool.tile([128, 3, TT], f32, tag="m1", name="m1")
                        state["z"] = rp_pool.tile([128, 3, TT], f32, tag="z", name="z")
                        state["m2p"] = rp_pool.tile([128, 3, TT], f32, tag="m2p", name="m2p")
                    m1_all, z_all = state["m1"], state["z"]
                    wsrc, e0, e1, nrows = [
                        (wq_s, 0, 128, 128),
                        (wq_s, 128, 256, 128),
                        (wkv_s, 0, 2 * HD, 64),
                    ][ri]
                    ps = ps_mm.tile([128, TT], f32, tag="mm2k", name=f"pj{ri}")
                    for idb in range(NDB):
                        nc.tensor.matmul(
                            ps,
                            wsrc[:, idb, e0:e1],
                            htile[:, idb, :],
                            start=(idb == 0),
                            stop=(idb == NDB - 1),
                        )
                    if ri == 2:
                        # stage V to SBUF right away (ACT) so the V
                        # transposes don't wait on the DVE rope muls
                        v_sb = vs_pool.tile([128, TT], f32, tag="v_sb")
                        nc.scalar.copy(out=v_sb[64:128, :], in_=ps[64:128, :])
                        state["v_sb"] = v_sb
                    # RoPE input products; m2p (swapped z) comes via DMA
                    nc.vector.tensor_mul(
                        m1_all[:nrows, ri, :], ps[:nrows], cos_s[:nrows, tsl]
                    )
                    nc.vector.tensor_mul(
                        z_all[:nrows, ri, :], ps[:nrows], sinp_s[:nrows, tsl]
                    )

                def proj_tail(it, state):
                    """Swap-DMA + RoPE adds + V transpose for token tile it."""
                    t0 = it * TT
                    tsl = slice(t0, t0 + TT)
                    m1_all, z_all, m2p_all = state["m1"], state["z"], state["m2p"]
                    # partition swap of z (32-row block pairs 0<->1, 2<->3)
                    for c, lo in ((0, 32), (1, 0), (2, 96), (3, 64)):
                        nc.sync.dma_start(
                            out=m2p_all[c * 32 : c * 32 + 32, :, :],
                            in_=z_all[lo : lo + 32, :, :],
                        )
                    # rope adds; k lands twice so odd q-heads can matmul
                    # from partition base 64 (tile_position row packing)
                    nc.vector.tensor_add(
                        kt_s[0:64, tsl], m1_all[0:64, 2, :], m2p_all[0:64, 2, :]
                    )
                    nc.vector.tensor_add(
                        kt_s[64:128, tsl], m1_all[0:64, 2, :], m2p_all[0:64, 2, :]
                    )
                    nc.vector.tensor_add(
                        qt_s[0][:, tsl], m1_all[:, 0, :], m2p_all[:, 0, :]
                    )
                    nc.vector.tensor_add(
                        qt_s[1][:, tsl], m1_all[:, 1, :], m2p_all[:, 1, :]
                    )
                    # V: [d, t] -> [t, d] via PE transpose (V was staged
                    # to SBUF right after the kv projection)
                    v_sb = state["v_sb"]
                    for c4 in range(TT // 128):
                        vt_ps = ps_mm.tile([128, HD], f32, tag="mm2k", name="vt")
                        nc.tensor.transpose(
                            vt_ps,
                            v_sb[64:128, c4 * 128 : (c4 + 1) * 128],
                            ident[64:128, 64:128],
                        )
                        nc.vector.tensor_copy(
                            out=va_s[:, it * 4 + c4, 0:HD], in_=vt_ps
                        )

                def attn_block(it, astate, rp, kb, nkb):
                    """One 128-key attention block of query tile it."""
                    b, iq = it // 4, it % 4
                    q0 = iq * QT
                    bq = b * S + q0
                    qtile = qt_s[rp]
                    if kb == 0:
                        # 2 PSUM banks shared by both head-pairs: rp1's first
                        # A@V slot-waits on rp0's PSUM evacuation copies
                        astate[f"o{rp}"] = [
                            ps_o.tile(
                                [65, QT], f32, tag=f"o{i}", name=f"o{i}", bufs=1
                            )
                            for i in range(2)
                        ]
                    o_ps = astate[f"o{rp}"]
                    ksl = slice(b * S + kb * KBLK, b * S + (kb + 1) * KBLK)
                    j = kb - q0 // KBLK
                    # query-column truncation: straddle block j only
                    # touches queries f >= 128*j
                    c0 = 128 * j if (causal and j > 0) else 0
                    s_ps = ps_s.tile([128, 2, QT], f32, tag="s")
                    pt = pt_pool.tile([128, 2, QT], bf16, tag="pt")
                    for h in range(2):
                        hb = h * 64
                        nc.tensor.matmul(
                            s_ps[:, h, c0:QT],
                            kt_s[hb : hb + 64, ksl],
                            qtile[hb : hb + 64, bq + c0 : bq + QT],
                            start=True,
                            stop=True,
                        )
                    if causal:
                        nc.scalar.activation(
                            pt[:, :, c0:QT], s_ps[:, :, c0:QT], Exp, scale=0.125
                        )
                    else:
                        mk = pt_pool.tile([128, QT], f32, tag="mk")
                        sm = pt_pool.tile([128, 2, QT], f32, tag="sm")
                        nc.sync.dma_start(
                            out=mk,
                            in_=maskT[kb * KBLK : (kb + 1) * KBLK, q0 : q0 + QT],
                        )
                        for h in range(2):
                            nc.vector.scalar_tensor_tensor(
                                out=sm[:, h, :],
                                in0=s_ps[:, h, :],
                                scalar=0.125,
                                in1=mk,
                                op0=mybir.AluOpType.mult,
                                op1=mybir.AluOpType.add,
                            )
                        nc.scalar.activation(pt, sm, Exp, scale=1.0)
                    for h in range(2):
                        if causal and j >= 0:
                            # zero the sub-diagonal triangle in place on the
                            # (otherwise idle) gpsimd: keep where (f-c0) >= p
                            nc.gpsimd.affine_select(
                                out=pt[:, h, c0:QT],
                                in_=pt[:, h, c0:QT],
                                compare_op=mybir.AluOpType.is_ge,
                                fill=0.0,
                                base=0,
                                channel_multiplier=-1,
                                pattern=[[1, QT - c0]],
                            )
                        nc.tensor.matmul(
                            o_ps[h][:, c0:QT],
                            va_s[:, b * (S // 128) + kb, :],
                            pt[:, h, c0:QT],
                            start=(kb == 0),
                            stop=(kb == nkb - 1),
                        )

                def evac_rp(astate, rp):
                    """Evacuate the pair's A@V accumulators (frees the two
                    o PSUM banks for the next head pair)."""
                    ou_all = astate["ou"]
                    o_ps = astate[f"o{rp}"]
                    for h in range(2):
                        nc.vector.tensor_copy(
                            out=ou_all[:, rp * 2 + h, :], in_=o_ps[h]
                        )

                def normalize_tail(astate):
                    """Batched softmax normalization for all 4 heads: the
                    denominator rows bounce through a [32, 64] layout so
                    reciprocal uses 32 lanes, and the partition broadcast is
                    a stride-0 DMA through a DRAM scratch (no engine time)."""
                    on_t, ou_all = astate["on_t"], astate["ou"]
                    r32 = nm_pool.tile([32, 64], f32, tag="r32")
                    nc.sync.dma_start(out=r32, in_=ou_all[64:65, :, :])
                    r32r = nm_pool.tile([32, 64], f32, tag="r32r")
                    nc.vector.reciprocal(r32r, r32)
                    rd = dr_pool.tile([1, 4 * QT], f32, tag="rd", name="rd")
                    nc.sync.dma_start(out=rd, in_=r32r)
                    rec_b = nm_pool.tile([64, 4 * QT], f32, tag="rb")
                    nc.sync.dma_start(
                        out=rec_b, in_=rd.partition_broadcast(64)[:, 0, :]
                    )
                    for rp in range(2):
                        for h in range(2):
                            hh = rp * 2 + h
                            nc.vector.tensor_mul(
                                on_t[rp][h * 64 : h * 64 + 64, :],
                                ou_all[0:64, hh, :],
                                rec_b[:, hh * QT : (hh + 1) * QT],
                            )

                def attn_begin(it):
                    b, iq = it // 4, it % 4
                    nkb = (iq * QT // KBLK + 4) if causal else (S // KBLK)
                    astate = {
                        "on_t": [
                            on_pool.tile(
                                [128, QT], bf16, tag=f"on{i}", name=f"on{i}"
                            )
                            for i in range(2)
                        ],
                        "ou": nm_pool.tile([65, 4, QT], f32, tag="ou", name="ou", bufs=2),
                        "nkb": nkb,
                    }
                    return astate

                def attn_out(it, on_t):
                    """Wo projection + coalesced bf16 output store."""
                    b, iq = it // 4, it % 4
                    q0 = iq * QT
                    qsl = slice(b * S + q0, b * S + q0 + QT)
                    out_acc = oa_pool.tile([128, D // 128, QT], bf16, tag="oacc")
                    for eb in range(D // 128):
                        wo_ps = ps_mm.tile([128, QT], f32, tag="mm2k", name="wo")
                        for db in range(2):
                            nc.tensor.matmul(
                                wo_ps,
                                wo_s[:, db, eb * 128 : (eb + 1) * 128],
                                on_t[db],
                                start=(db == 0),
                                stop=(db == 1),
                            )
                        # split PSUM evacuation: mostly DVE, a bit on ACT
                        if eb % 3 == 1:
                            nc.scalar.copy(out=out_acc[:, eb, :], in_=wo_ps)
                        else:
                            nc.vector.tensor_copy(
                                out=out_acc[:, eb, :], in_=wo_ps
                            )
                    half = D // 256
                    nc.sync.dma_start(
                        out=outT_b3[:, 0:half, qsl], in_=out_acc[:, 0:half, :]
                    )
                    nc.sync.dma_start(
                        out=outT_b3[:, half:, qsl], in_=out_acc[:, half:, :]
                    )

                # ---------------- the software-pipelined merged loop:
                # attention for tile s-1, projection for tile s, Wo for tile
                # s-2.  The 2-step Wo skew means the Wo matmuls' inputs are
                # always long-ready (they fill PE gaps, and their PSUM
                # evacuation copies never head-of-line-block the exps), and
                # the normalize chain of s-1 has a full step to complete.
                htiles = {0: prefetch(0), 1: prefetch(1)}
                on_hist = {}
                for step in range(NT + 2):
                    if step + 2 <= NT - 1:
                        htiles[step + 2] = prefetch(step + 2)
                    if 1 <= step <= NT:
                        astate = attn_begin(step - 1)
                        nkb = astate["nkb"]
                        for rp in range(2):
                            for kb in range(nkb):
                                attn_block(step - 1, astate, rp, kb, nkb)
                            evac_rp(astate, rp)
                        normalize_tail(astate)
                        on_hist[step - 1] = astate["on_t"]
                    if step <= NT - 1:
                        pstate = {}
                        htile = htiles.pop(step)
                        for ri in range(3):
                            proj_chunk(step, htile, ri, pstate)
                        proj_tail(step, pstate)
                    if step >= 2:
                        attn_out(step - 2, on_hist.pop(step - 2))

    nc.compile()
    return nc


def _host_inputs(inputs, causal):
    """Shard + transpose the full inputs into 8 per-core input maps."""
    h = np.asarray(inputs["hidden_states"], np.float32)
    cos = np.asarray(inputs["position_cos"], np.float32)
    sin = np.asarray(inputs["position_sin"], np.float32)
    Wq = np.asarray(inputs["Wq"], np.float32)
    Wk = np.asarray(inputs["Wk"], np.float32)
    Wv = np.asarray(inputs["Wv"], np.float32)
    Wo = np.asarray(inputs["Wo"], np.float32)
    mask = np.asarray(inputs["attention_mask"], np.float32)[0, 0]

    hT = np.ascontiguousarray(h.reshape(T, D).T).astype(BF16)

    cosT = np.tile(cos.T, (1, B))                     # [64, T]
    sinT = np.tile(sin.T, (1, B))
    cos2 = np.ascontiguousarray(np.vstack([cosT, cosT]).astype(np.float32))
    s_signed = np.vstack([-sinT[0:32], sinT[32:64]])  # rot_half sign baked in
    sin2s = np.vstack([s_signed, s_signed])           # [128, T]
    # pre-swap so that z[p] = x[p]*sinp[p]; m2[p] = z[swap(p)] equals
    # rot_half(x)[p] * sin_signed[p]  (swap = 32-row block pairs 0<->1, 2<->3)
    swap_idx = np.concatenate(
        [np.arange(32, 64), np.arange(0, 32), np.arange(96, 128), np.arange(64, 96)]
    )
    sinp = np.ascontiguousarray(sin2s[swap_idx].astype(np.float32))

    maskT = np.ascontiguousarray(mask.T).astype(np.float32)

    in_maps = []
    for g in range(8):
        in_maps.append(
            {
                "hT": hT,
                "wqT": np.ascontiguousarray(
                    Wq[g * EQ : (g + 1) * EQ].T
                ).astype(BF16),
                "wkvT": np.ascontiguousarray(
                    np.concatenate(
                        [
                            Wk[g * HD : (g + 1) * HD].T,
                            Wv[g * HD : (g + 1) * HD].T,
                        ],
                        axis=1,
                    )
                ).astype(BF16),
                "woT": np.ascontiguousarray(
                    Wo[:, g * EQ : (g + 1) * EQ].T
                ).astype(BF16),
                "cos2": cos2,
                "sinp": sinp,
                "maskT": maskT,
            }
        )
    return in_maps


def _is_causal(mask):
    m = np.asarray(mask, np.float32)[0, 0]
    tri = np.tril(np.ones((S, S), bool))
    return bool(np.all(m[tri] == 0.0) and np.all(m[~tri] <= -1e8))


def _assemble(results):
    acc = np.zeros((D, T), np.float32)
    for r in results:
        acc += r["outT"].astype(np.float32)
    return np.ascontiguousarray(acc.reshape(D, B, S).transpose(1, 2, 0))


def kernel(**inputs) -> np.ndarray:
    from concourse.bass_utils import run_bass_kernel_spmd

    causal = _is_causal(inputs["attention_mask"])
    key = ("prog", causal)
    if key not in _CACHE:
        _CACHE[key] = _build_program(causal)
    nc = _CACHE[key]

    in_maps = _host_inputs(inputs, causal)
    res = run_bass_kernel_spmd(nc, in_maps, core_ids=list(range(8)))
    return _assemble(res.results)


# revision 36
# speedup vs baseline: 1.1685x; 1.0020x over previous
"""Trainium2 Bass kernel for GQA multi-head attention (nn_MultiHeadAttention).

Reference computation (fp32):
    q = h @ Wq^T -> RoPE ; k = h @ Wk^T -> RoPE ; v = h @ Wv^T
    scores = q k^T / sqrt(64) + causal_mask ; w = softmax(scores)
    out = (w v) @ Wo^T

Shapes: h [2,2048,2048], Wq [2048,2048], Wk/Wv [512,2048], Wo [2048,2048],
32 q heads / 8 kv heads (GQA group=4), head_dim 64.

Sharding: tensor-parallel over the 8 kv-head groups, one group per core.
Core g owns q heads [4g,4g+4), kv head g, Wo columns [256g, 256(g+1)).
Each core computes a full-token partial of the output projection; the host
sums the 8 partials (the Wo contraction splits over head blocks).

Structure: a single software-pipelined loop over the 8 token tiles of 512;
at step s it emits [hT prefetch for s+1] [attention for query tile s-1]
[QKV projection + RoPE for tile s] [Wo projection + output store for tile
s-2].  The 2-step Wo skew means the Wo matmuls' inputs are always
long-ready (they fill PE gaps so the HAM clock gate stays warm, and their
PSUM-evacuation copies never head-of-line-block the exps), and the softmax
normalize chain of tile s-1 gets a full step of slack before anything
consumes it.  Engine assignment keeps each FIFO's latency-critical ops
away from bulk work: PE matmuls; ACT exps + a slice of the PSUM
evacuations; DVE RoPE muls/adds + evacuation copies + reciprocal; GPSIMD
causal masks only; the normalize partition-broadcast is a stride-0 DMA
through a DRAM scratch (no compute engine involved).

Everything is kept transposed: h^T [2048, 4096] comes in, Q^T/K^T [d, t]
fall out of the projections directly, scores are S^T[k, q], softmax is a
plain exp (scores are O(5), fp32-safe) with causal-skip at 128-key-block
granularity AND 128-query-column truncation inside the diagonal straddle
blocks (scores / exp / mask / A@V all skip the dead triangle).  A@V uses V
augmented with a ones-column so softmax denominators fall out of the same
matmul.  RoPE's rot_half partition swap runs on a pre-scaled copy
(z = x * sin_pre, then swap z via SBUF DMA) so no scalar-engine staging
copy is needed; the odd-head K replica is a second (partition-shifted)
DVE add instead of a DMA.  Denominator reciprocals for all 4 heads of a
query tile are batched through one [32, 64] bounce so the DVE reciprocal
uses 32 lanes.  Output partials are stored bf16 (halves the HBM write) as
one coalesced DMA per query tile.
"""

import sys

for _p in ("/opt/trn_rl_repo",):
    if _p not in sys.path:
        sys.path.insert(0, _p)

import numpy as np
import ml_dtypes

D = 2048          # model dim
HD = 64           # head dim
S = 2048          # sequence
B = 2             # batch
T = B * S         # total tokens
EQ = 256          # q-projection rows per core (4 heads x 64)
TT = 512          # token tile (both projection and query tile)
NT = T // TT      # 8 merged steps
NDB = D // 128    # contraction blocks for projections
QT = 512          # query tile for attention
KBLK = 128        # key block for attention
BF16 = ml_dtypes.bfloat16

_CACHE = {}


def _build_program(causal: bool):
    """Build the single-core Bass/Tile program (identical across cores)."""
    import concourse.bass as bass
    import concourse.mybir as mybir
    import concourse.tile as tile
    from concourse import bacc
    from concourse.masks import make_identity

    f32 = mybir.dt.float32
    bf16 = mybir.dt.bfloat16
    f8 = mybir.dt.float8e4

    nc = bacc.Bacc("TRN2", target_bir_lowering=False, debug=False)

    hT = nc.dram_tensor("hT", [D, T], bf16, kind="ExternalInput").ap()
    wqT = nc.dram_tensor("wqT", [D, EQ], bf16, kind="ExternalInput").ap()
    # k and v projection weights packed [D, 64+64] so one matmul produces both
    wkvT = nc.dram_tensor("wkvT", [D, 2 * HD], bf16, kind="ExternalInput").ap()
    woT = nc.dram_tensor("woT", [EQ, D], bf16, kind="ExternalInput").ap()
    cos2 = nc.dram_tensor("cos2", [128, T], f32, kind="ExternalInput").ap()
    # sin with rot_half sign AND partition swap pre-applied (see _host_inputs)
    sinp = nc.dram_tensor("sinp", [128, T], f32, kind="ExternalInput").ap()
    # mask^T tiles, only used when causal=False
    maskT = nc.dram_tensor("maskT", [S, S], f32, kind="ExternalInput").ap()
    outT = nc.dram_tensor("outT", [D, T], bf16, kind="ExternalOutput").ap()

    hT_b3 = hT.rearrange("(n p) t -> p n t", p=128)     # [128, 16, T]
    wqT_b = wqT.rearrange("(n p) e -> p n e", p=128)
    wkvT_b = wkvT.rearrange("(n p) e -> p n e", p=128)
    woT_b = woT.rearrange("(n p) e -> p n e", p=128)
    outT_b3 = outT.rearrange("(n p) t -> p n t", p=128)  # [128, 16, T]

    Exp = mybir.ActivationFunctionType.Exp
    PSUM = bass.MemorySpace.PSUM

    with tile.TileContext(nc) as tc:
        import contextlib

        with contextlib.ExitStack() as stack:
            const = stack.enter_context(tc.tile_pool(name="const", bufs=1))

            wq_s = const.tile([128, NDB, EQ], bf16)
            wkv_s = const.tile([128, NDB, 2 * HD], bf16)
            wo_s = const.tile([128, 2, D], bf16)
            cos_s = const.tile([128, T], f32)
            sinp_s = const.tile([128, T], f32)
            qt_s = [
                const.tile([128, T], bf16, tag=f"qt{i}", name=f"qt{i}")
                for i in range(2)
            ]
            kt_s = const.tile([128, T], bf16)
            va_s = const.tile([128, T // 128, HD + 1], bf16)
            tri_s = const.tile([128, 4, QT], bf16)
            ident = const.tile([128, 128], f32)

            nc.sync.dma_start(out=wq_s, in_=wqT_b)
            nc.sync.dma_start(out=wkv_s, in_=wkvT_b)
            # cos/sin aren't needed until the first RoPE (~10us after the
            # first matmuls) and wo not until step 2: load them on the
            # scalar HWDGE ring so the sync ring goes straight from the
            # projection weights to the first two hT tiles
            nc.scalar.dma_start(out=cos_s, in_=cos2)
            nc.scalar.dma_start(out=sinp_s, in_=sinp)
            nc.scalar.dma_start(out=wo_s, in_=woT_b)

            make_identity(nc, ident)
            # ones column of the augmented V
            nc.gpsimd.memset(va_s[:, :, HD : HD + 1], 1.0)
            # multiplicative causal masks for the 4 straddle offsets:
            # tri_s[p, j, f] = 1.0 where f >= p + 128*j else 0.0
            for j in range(4):
                nc.gpsimd.memset(tri_s[:, j, :], 1.0)
                nc.gpsimd.affine_select(
                    out=tri_s[:, j, :],
                    in_=tri_s[:, j, :],
                    compare_op=mybir.AluOpType.is_ge,
                    fill=0.0,
                    base=-128 * j,
                    channel_multiplier=-1,
                    pattern=[[1, QT]],
                )

            # ---------------- pools for the merged pipeline
            with contextlib.ExitStack() as pp:
                ht_pool = pp.enter_context(tc.tile_pool(name="ht", bufs=3))
                # m1 / z / swapped-z rope scratch, all three ropes stacked
                rp_pool = pp.enter_context(tc.tile_pool(name="rp", bufs=1))
                vs_pool = pp.enter_context(tc.tile_pool(name="vs", bufs=2))
                # shared-PSUM pool: proj accumulators, V transposes, Wo tiles
                ps_mm = pp.enter_context(
                    tc.tile_pool(name="ps_mm", bufs=2, space=PSUM)
                )
                ps_s = pp.enter_context(
                    tc.tile_pool(name="ps_s", bufs=2, space=PSUM)
                )
                ps_o = pp.enter_context(
                    tc.tile_pool(name="ps_o", bufs=1, space=PSUM)
                )
                pt_pool = pp.enter_context(tc.tile_pool(name="pt", bufs=4))
                on_pool = pp.enter_context(tc.tile_pool(name="on", bufs=2))
                nm_pool = pp.enter_context(tc.tile_pool(name="nm", bufs=1))
                oa_pool = pp.enter_context(tc.tile_pool(name="oa", bufs=1))
                dr_pool = pp.enter_context(
                    tc.tile_pool(name="dr", bufs=2, space="DRAM")
                )

                def prefetch(it):
                    htile = ht_pool.tile([128, NDB, TT], bf16, tag="ht",
                                         name=f"ht{it}")
                    t0 = it * TT
                    nc.sync.dma_start(out=htile, in_=hT_b3[:, :, t0 : t0 + TT])
                    return htile

                def proj_chunk(it, htile, ri, state):
                    """One projection group (q01 / q23 / kv) + its RoPE muls."""
                    t0 = it * TT
                    tsl = slice(t0, t0 + TT)
                    if ri == 0:
                        state["m1"] = rp_pool.tile([128, 3, TT], f32, tag="m1", name="m1")
                        state["z"] = rp_pool.tile([128, 3, TT], f32, tag="z", name="z")
                        state["m2p"] = rp_pool.tile([128, 3, TT], f32, tag="m2p", name="m2p")
                    m1_all, z_all = state["m1"], state["z"]
                    wsrc, e0, e1, nrows = [
                        (wq_s, 0, 128, 128),
                        (wq_s, 128, 256, 128),
                        (wkv_s, 0, 2 * HD, 64),
                    ][ri]
                    ps = ps_mm.tile([128, TT], f32, tag="mm2k", name=f"pj{ri}")
                    for idb in range(NDB):
                        nc.tensor.matmul(
                            ps,
                            wsrc[:, idb, e0:e1],
                            htile[:, idb, :],
                            start=(idb == 0),
                            stop=(idb == NDB - 1),
                        )
                    if ri == 2:
                        # stage V to SBUF right away (ACT) so the V
                        # transposes don't wait on the DVE rope muls
                        v_sb = vs_pool.tile([128, TT], f32, tag="v_sb")
                        nc.scalar.copy(out=v_sb[64:128, :], in_=ps[64:128, :])
                        state["v_sb"] = v_sb
                    # RoPE input products; m2p (swapped z) comes via DMA
                    nc.vector.tensor_mul(
                        m1_all[:nrows, ri, :], ps[:nrows], cos_s[:nrows, tsl]
                    )
                    nc.vector.tensor_mul(
                        z_all[:nrows, ri, :], ps[:nrows], sinp_s[:nrows, tsl]
                    )

                def proj_tail(it, state):
                    """Swap-DMA + RoPE adds + V transpose for token tile it."""
                    t0 = it * TT
                    tsl = slice(t0, t0 + TT)
                    m1_all, z_all, m2p_all = state["m1"], state["z"], state["m2p"]
                    # partition swap of z (32-row block pairs 0<->1, 2<->3)
                    for c, lo in ((0, 32), (1, 0), (2, 96), (3, 64)):
                        nc.sync.dma_start(
                            out=m2p_all[c * 32 : c * 32 + 32, :, :],
                            in_=z_all[lo : lo + 32, :, :],
                        )
                    # rope adds; k lands twice so odd q-heads can matmul
                    # from partition base 64 (tile_position row packing)
                    nc.vector.tensor_add(
                        kt_s[0:64, tsl], m1_all[0:64, 2, :], m2p_all[0:64, 2, :]
                    )
                    nc.vector.tensor_add(
                        kt_s[64:128, tsl], m1_all[0:64, 2, :], m2p_all[0:64, 2, :]
                    )
                    nc.vector.tensor_add(
                        qt_s[0][:, tsl], m1_all[:, 0, :], m2p_all[:, 0, :]
                    )
                    nc.vector.tensor_add(
                        qt_s[1][:, tsl], m1_all[:, 1, :], m2p_all[:, 1, :]
                    )
                    # V: [d, t] -> [t, d] via PE transpose (V was staged
                    # to SBUF right after the kv projection)
                    v_sb = state["v_sb"]
                    for c4 in range(TT // 128):
                        vt_ps = ps_mm.tile([128, HD], f32, tag="mm2k", name="vt")
                        nc.tensor.transpose(
                            vt_ps,
                            v_sb[64:128, c4 * 128 : (c4 + 1) * 128],
                            ident[64:128, 64:128],
                        )
                        nc.vector.tensor_copy(
                            out=va_s[:, it * 4 + c4, 0:HD], in_=vt_ps
                        )

                def attn_block(it, astate, rp, kb, nkb):
                    """One 128-key attention block of query tile it."""
                    b, iq = it // 4, it % 4
                    q0 = iq * QT
                    bq = b * S + q0
                    qtile = qt_s[rp]
                    if kb == 0:
                        # 2 PSUM banks shared by both head-pairs: rp1's first
                        # A@V slot-waits on rp0's PSUM evacuation copies
                        astate[f"o{rp}"] = [
                            ps_o.tile(
                                [65, QT], f32, tag=f"o{i}", name=f"o{i}", bufs=1
                            )
                            for i in range(2)
                        ]
                    o_ps = astate[f"o{rp}"]
                    ksl = slice(b * S + kb * KBLK, b * S + (kb + 1) * KBLK)
                    j = kb - q0 // KBLK
                    # query-column truncation: straddle block j only
                    # touches queries f >= 128*j
                    c0 = 128 * j if (causal and j > 0) else 0
                    s_ps = ps_s.tile([128, 2, QT], f32, tag="s")
                    pt = pt_pool.tile([128, 2, QT], bf16, tag="pt")
                    for h in range(2):
                        hb = h * 64
                        nc.tensor.matmul(
                            s_ps[:, h, c0:QT],
                            kt_s[hb : hb + 64, ksl],
                            qtile[hb : hb + 64, bq + c0 : bq + QT],
                            start=True,
                            stop=True,
                        )
                    if causal:
                        nc.scalar.activation(
                            pt[:, :, c0:QT], s_ps[:, :, c0:QT], Exp, scale=0.125
                        )
                    else:
                        mk = pt_pool.tile([128, QT], f32, tag="mk")
                        sm = pt_pool.tile([128, 2, QT], f32, tag="sm")
                        nc.sync.dma_start(
                            out=mk,
                            in_=maskT[kb * KBLK : (kb + 1) * KBLK, q0 : q0 + QT],
                        )
                        for h in range(2):
                            nc.vector.scalar_tensor_tensor(
                                out=sm[:, h, :],
                                in0=s_ps[:, h, :],
                                scalar=0.125,
                                in1=mk,
                                op0=mybir.AluOpType.mult,
                                op1=mybir.AluOpType.add,
                            )
                        nc.scalar.activation(pt, sm, Exp, scale=1.0)
                    for h in range(2):
                        if causal and j >= 0:
                            # zero the sub-diagonal triangle in place on the
                            # (otherwise idle) gpsimd: keep where (f-c0) >= p
                            nc.gpsimd.affine_select(
                                out=pt[:, h, c0:QT],
                                in_=pt[:, h, c0:QT],
                                compare_op=mybir.AluOpType.is_ge,
                                fill=0.0,
                                base=0,
                                channel_multiplier=-1,
                                pattern=[[1, QT - c0]],
                            )
                        nc.tensor.matmul(
                            o_ps[h][:, c0:QT],
                            va_s[:, b * (S // 128) + kb, :],
                            pt[:, h, c0:QT],
                            start=(kb == 0),
                            stop=(kb == nkb - 1),
                        )

                def evac_rp(astate, rp):
                    """Evacuate the pair's A@V accumulators (frees the two
                    o PSUM banks for the next head pair)."""
                    ou_all = astate["ou"]
                    o_ps = astate[f"o{rp}"]
                    for h in range(2):
                        nc.vector.tensor_copy(
                            out=ou_all[:, rp * 2 + h, :], in_=o_ps[h]
                        )

                def normalize_tail(astate):
                    """Batched softmax normalization for all 4 heads: the
                    denominator rows bounce through a [32, 64] layout so
                    reciprocal uses 32 lanes, and the partition broadcast is
                    a stride-0 DMA through a DRAM scratch (no engine time)."""
                    on_t, ou_all = astate["on_t"], astate["ou"]
                    r32 = nm_pool.tile([32, 64], f32, tag="r32")
                    nc.sync.dma_start(out=r32, in_=ou_all[64:65, :, :])
                    r32r = nm_pool.tile([32, 64], f32, tag="r32r")
                    nc.vector.reciprocal(r32r, r32)
                    rd = dr_pool.tile([1, 4 * QT], f32, tag="rd", name="rd")
                    nc.sync.dma_start(out=rd, in_=r32r)
                    rec_b = nm_pool.tile([64, 4 * QT], f32, tag="rb")
                    nc.sync.dma_start(
                        out=rec_b, in_=rd.partition_broadcast(64)[:, 0, :]
                    )
                    for rp in range(2):
                        for h in range(2):
                            hh = rp * 2 + h
                            nc.vector.tensor_mul(
                                on_t[rp][h * 64 : h * 64 + 64, :],
                                ou_all[0:64, hh, :],
                                rec_b[:, hh * QT : (hh + 1) * QT],
                            )

                def attn_begin(it):
                    b, iq = it // 4, it % 4
                    nkb = (iq * QT // KBLK + 4) if causal else (S // KBLK)
                    astate = {
                        "on_t": [
                            on_pool.tile(
                                [128, QT], bf16, tag=f"on{i}", name=f"on{i}"
                            )
                            for i in range(2)
                        ],
                        "ou": nm_pool.tile([65, 4, QT], f32, tag="ou", name="ou", bufs=2),
                        "nkb": nkb,
                    }
                    return astate

                def attn_out(it, on_t):
                    """Wo projection + coalesced bf16 output store."""
                    b, iq = it // 4, it % 4
                    q0 = iq * QT
                    qsl = slice(b * S + q0, b * S + q0 + QT)
                    out_acc = oa_pool.tile([128, D // 128, QT], bf16, tag="oacc")
                    for eb in range(D // 128):
                        wo_ps = ps_mm.tile([128, QT], f32, tag="mm2k", name="wo")
                        for db in range(2):
                            nc.tensor.matmul(
                                wo_ps,
                                wo_s[:, db, eb * 128 : (eb + 1) * 128],
                                on_t[db],
                                start=(db == 0),
                                stop=(db == 1),
                            )
                        # split PSUM evacuation: mostly DVE, a bit on ACT
                        if eb % 3 == 1:
                            nc.scalar.copy(out=out_acc[:, eb, :], in_=wo_ps)
                        else:
                            nc.vector.tensor_copy(
                                out=out_acc[:, eb, :], in_=wo_ps
                            )
                    half = D // 256
                    nc.sync.dma_start(
                        out=outT_b3[:, 0:half, qsl], in_=out_acc[:, 0:half, :]
                    )
                    nc.sync.dma_start(
                        out=outT_b3[:, half:, qsl], in_=out_acc[:, half:, :]
                    )

                # ---------------- the software-pipelined merged loop:
                # attention for tile s-1, projection for tile s, Wo for tile
                # s-2.  The 2-step Wo skew means the Wo matmuls' inputs are
                # always long-ready (they fill PE gaps, and their PSUM
                # evacuation copies never head-of-line-block the exps), and
                # the normalize chain of s-1 has a full step to complete.
                htiles = {0: prefetch(0), 1: prefetch(1)}
                on_hist = {}
                for step in range(NT + 2):
                    if step + 2 <= NT - 1:
                        htiles[step + 2] = prefetch(step + 2)
                    if 1 <= step <= NT:
                        astate = attn_begin(step - 1)
                        nkb = astate["nkb"]
                        for rp in range(2):
                            for kb in range(nkb):
                                attn_block(step - 1, astate, rp, kb, nkb)
                            evac_rp(astate, rp)
                        normalize_tail(astate)
                        on_hist[step - 1] = astate["on_t"]
                    if step <= NT - 1:
                        pstate = {}
                        htile = htiles.pop(step)
                        for ri in range(3):
                            proj_chunk(step, htile, ri, pstate)
                        proj_tail(step, pstate)
                    if step >= 2:
                        attn_out(step - 2, on_hist.pop(step - 2))

    nc.compile()
    return nc


def _host_inputs(inputs, causal):
    """Shard + transpose the full inputs into 8 per-core input maps."""
    h = np.asarray(inputs["hidden_states"], np.float32)
    cos = np.asarray(inputs["position_cos"], np.float32)
    sin = np.asarray(inputs["position_sin"], np.float32)
    Wq = np.asarray(inputs["Wq"], np.float32)
    Wk = np.asarray(inputs["Wk"], np.float32)
    Wv = np.asarray(inputs["Wv"], np.float32)
    Wo = np.asarray(inputs["Wo"], np.float32)
    mask = np.asarray(inputs["attention_mask"], np.float32)[0, 0]

    hT = np.ascontiguousarray(h.reshape(T, D).T).astype(BF16)

    cosT = np.tile(cos.T, (1, B))                     # [64, T]
    sinT = np.tile(sin.T, (1, B))
    cos2 = np.ascontiguousarray(np.vstack([cosT, cosT]).astype(np.float32))
    s_signed = np.vstack([-sinT[0:32], sinT[32:64]])  # rot_half sign baked in
    sin2s = np.vstack([s_signed, s_signed])           # [128, T]
    # pre-swap so that z[p] = x[p]*sinp[p]; m2[p] = z[swap(p)] equals
    # rot_half(x)[p] * sin_signed[p]  (swap = 32-row block pairs 0<->1, 2<->3)
    swap_idx = np.concatenate(
        [np.arange(32, 64), np.arange(0, 32), np.arange(96, 128), np.arange(64, 96)]
    )
    sinp = np.ascontiguousarray(sin2s[swap_idx].astype(np.float32))

    maskT = np.ascontiguousarray(mask.T).astype(np.float32)

    in_maps = []
    for g in range(8):
        in_maps.append(
            {
                "hT": hT,
                "wqT": np.ascontiguousarray(
                    Wq[g * EQ : (g + 1) * EQ].T
                ).astype(BF16),
                "wkvT": np.ascontiguousarray(
                    np.concatenate(
                        [
                            Wk[g * HD : (g + 1) * HD].T,
                            Wv[g * HD : (g + 1) * HD].T,
                        ],
                        axis=1,
                    )
                ).astype(BF16),
                "woT": np.ascontiguousarray(
                    Wo[:, g * EQ : (g + 1) * EQ].T
                ).astype(BF16),
                "cos2": cos2,
                "sinp": sinp,
                "maskT": maskT,
            }
        )
    return in_maps


def _is_causal(mask):
    m = np.asarray(mask, np.float32)[0, 0]
    tri = np.tril(np.ones((S, S), bool))
    return bool(np.all(m[tri] == 0.0) and np.all(m[~tri] <= -1e8))


def _assemble(results):
    acc = np.zeros((D, T), np.float32)
    for r in results:
        acc += r["outT"].astype(np.float32)
    return np.ascontiguousarray(acc.reshape(D, B, S).transpose(1, 2, 0))


def kernel(**inputs) -> np.ndarray:
    from concourse.bass_utils import run_bass_kernel_spmd

    causal = _is_causal(inputs["attention_mask"])
    key = ("prog", causal)
    if key not in _CACHE:
        _CACHE[key] = _build_program(causal)
    nc = _CACHE[key]

    in_maps = _host_inputs(inputs, causal)
    res = run_bass_kernel_spmd(nc, in_maps, core_ids=list(range(8)))
    return _assemble(res.results)
